# revision 1
# baseline (speedup 1.0000x reference)
"""Trainium2 Bass kernel for the class-balanced supervised-contrastive loss.

Math (reference semantics, shift-invariant form with constant shift 10):
  l_ij = (f_i . g_j) / T,  T = 0.1, g = [features; centers; features_ood]
  E_ij = exp(l_ij - 10)
  S_i  = sum_{j != i} E_ij / (w_j - eq_ij)        (w_j = class count, eq = label match)
  P_i  = sum_{j != i} eq_ij (l_ij - 10)
  loss = -mean_i( P_i / K_i - log S_i ),  K_i = batch count of class t_i

Key identity: for matched columns j (eq_ij = 1) the class equals t_i, so w_j is a
per-row constant w*. The device therefore only needs, per row:
  A_i  = sum_j exp(10*(r_ij + bias1_j))      bias1_j = (ln(1/w_j) - 10)/10
  S2_i = sum_j eq_ij * E1_ij                 (E1 = the summand of A)
  S3_i = sum_j eq_ij * psum_ij               (psum = r + bias1)
  diag = psum_ii                             (self column, for exclusion terms)
Everything else is O(B) host math.

Matmuls: plain fp32 on the PE runs at 4 cycles/row; instead each operand is
split into bf16 hi+lo on the host and r is computed as
fh.gh + fl.gh + fh.gl (error ~2^-18, validated 1.4e-7 end-to-end), three
full-rate bf16 matmuls. The column bias enters as a K=2 matmul of ones
against the (bias_hi, bias_lo) row pair.

Device layout per core (rows sharded, 512 rows/core):
  psum[m,ch] [128,512] = ones2^T @ (b_h;b_l) + 3-term split fT_m^T @ gT_ch
  ACT: E1 = exp(10*psum) with accum_out -> A partial
  DVE: scalar_tensor_tensor (ta == t_i) * E1 / psum with accum_out -> S2/S3
"""

import ml_dtypes
import numpy as np

import concourse.bass as bass
import concourse.mybir as mybir
import concourse.tile as tile
from concourse.bass_utils import run_bass_kernel_spmd

NCORES = 8
C, TEMP = 1000, 0.1
B, BO, D = 4096, 4096, 512
N = B + C + BO              # 9192
NPAD = 9216                 # 18 * 512
PAD = NPAD - N
NCH = NPAD // 512           # 18 column chunks
EQCH = 10                   # chunks covering batch+centers (cols < 5120)
RPC = B // NCORES           # 512 rows per core
MT = RPC // 128             # 4 row tiles per core

F32 = mybir.dt.float32
BF16 = mybir.dt.bfloat16
ALU = mybir.AluOpType
AF = mybir.ActivationFunctionType
BFNP = ml_dtypes.bfloat16

_nc_cache = []

# This walrus build accepts only one sync-wait command per engine instruction.
# Move surplus waits onto standalone EventSemaphore instructions just before
# the affected instruction (same engine, so blocking semantics are identical).
_SPLIT_SKIP = ("InstEventSemaphore",)


def _split_multi_waits(nc):
    n = 0
    for f in nc.m.functions:
        for bb in f.blocks:
            new = []
            for ins in bb.instructions:
                si = ins.sync_info
                if (
                    si is not None
                    and si.on_wait
                    and len(si.on_wait) > 1
                    and type(ins).__name__ not in _SPLIT_SKIP
                ):
                    waits = list(si.on_wait)
                    for w in waits[:-1]:
                        es = mybir.InstEventSemaphore(
                            name=f"wsplit_{n}",
                            engine=ins.engine,
                            sync_info=mybir.SyncInfo(on_wait=[w], on_update=[]),
                        )
                        n += 1
                        new.append(es)
                    ins.sync_info = mybir.SyncInfo(
                        on_wait=[waits[-1]], on_update=list(si.on_update)
                    )
                new.append(ins)
            bb.instructions = new
    return n


def _build_nc(eqw=EQCH, corr_last=None, woff=None):
    nc = bass.Bass()
    # host pre-tiles to the SBUF layout: col chunk ch lives at [128, 2048]
    # block ch with inner offset 512*k + j  (k = contraction slice)
    gTh = nc.declare_dram_parameter("gTh", [128, NCH * 2048], BF16, isOutput=False)
    gTl = nc.declare_dram_parameter("gTl", [128, eqw * 2048], BF16, isOutput=False)
    fTh = nc.declare_dram_parameter("fTh", [128, 2048], BF16, isOutput=False)
    fTl = nc.declare_dram_parameter("fTl", [128, 2048], BF16, isOutput=False)
    # [2, .]: row 0 = (ones128, bias_hi row, bias_hi self), row 1 = lo parts
    cst = nc.declare_dram_parameter("cst", [2, 128 + NPAD + RPC], BF16, isOutput=False)
    ta = nc.declare_dram_parameter("ta", [128, eqw * 512], F32, isOutput=False)
    tvec = nc.declare_dram_parameter("tvec", [128, MT], F32, isOutput=False)
    ident = nc.declare_dram_parameter("ident", [128, 128], F32, isOutput=False)
    cw = nc.declare_dram_parameter("cw", [128, (NCH - eqw) * 512], F32, isOutput=False)
    out = nc.declare_dram_parameter("out", [128, 4 * MT], F32, isOutput=True)

    with tile.TileContext(nc) as tc:
        with (
            tc.tile_pool(name="const", bufs=1) as const,
            tc.tile_pool(name="stats", bufs=1) as stats,
            tc.tile_pool(name="gt", bufs=3) as gtp,
            tc.tile_pool(name="e1", bufs=3) as e1p,
            tc.tile_pool(name="scr", bufs=2) as scr,
            tc.tile_pool(name="psum", bufs=2, space="PSUM") as psp,
            tc.tile_pool(name="psum2", bufs=3, space="PSUM") as psp2,
        ):
            fth = const.tile([128, 4 * RPC], BF16)
            ftl = const.tile([128, 4 * RPC], BF16)
            for k in range(4):
                nc.sync.dma_start(
                    out=fth[:, 512 * k : 512 * (k + 1)],
                    in_=fTh[:, 512 * k : 512 * (k + 1)],
                )
            cst_sb = const.tile([2, 128 + NPAD + RPC], BF16)
            ta_sb = const.tile([128, eqw * 512], F32)
            tvec_sb = const.tile([128, MT], F32)
            ident_sb = const.tile([128, 128], F32)
            cw_sb = const.tile([128, (NCH - eqw) * 512], F32)
            ones_sb = cst_sb[:, 0:128]
            brow_sb = cst_sb[:, 128 : 128 + NPAD]
            bself_sb = cst_sb[:, 128 + NPAD : 128 + NPAD + RPC]

            outsb = stats.tile([128, 4 * MT], F32)
            ncheap = (NCH - eqw) // 2
            a_slot = [stats.tile([128, ncheap + eqw + 1], F32, name=f"a{m}") for m in range(MT)]
            s2_slot = [stats.tile([128, eqw], F32, name=f"s2{m}") for m in range(MT)]
            s3_slot = [stats.tile([128, eqw], F32, name=f"s3{m}") for m in range(MT)]

            def emit_group(ps, rhs_bias, rhs_h, rhs_l, m, full, corr=None):
                """bias + split matmul accumulation into psum ps.

                rhs_h/rhs_l are callables (k, n) -> AP of [128, n]. full=True
                emits the 3-term bf16 hi/lo split (error ~2^-18), with the two
                correction terms optionally narrowed to the first corr_n
                columns (where the matched columns live); full=False emits
                fh.gh only (per-dot error ~7e-5 random sign, fine for the OOD
                exp-sum). Order is fixed so the diag group matches the main
                loop bit-exactly.
                """
                if rhs_bias is not None:
                    nc.tensor.matmul(ps, ones_sb, rhs_bias, start=True, stop=False)
                terms = ((fth, rhs_h), (ftl, rhs_h), (fth, rhs_l)) if full else ((fth, rhs_h),)
                for ti, (lhs, rhs) in enumerate(terms):
                    off, n = (0, ps.shape[-1]) if (ti == 0 or corr is None) else corr
                    for k in range(4):
                        lsl = slice(512 * k + 128 * m, 512 * k + 128 * (m + 1))
                        nc.tensor.matmul(
                            ps[:, off : off + n],
                            lhs[:, lsl],
                            rhs(k, off, n),
                            start=(rhs_bias is None and ti == 0 and k == 0),
                            stop=(ti == len(terms) - 1 and k == 3),
                        )

            # Cheap chunks first: they need only fth/gth/cw, so the PE can
            # start while the remaining constants stream in. No bias matmul:
            # the per-column exp(10*bias) factor is applied by the DVE inside
            # the same fused reduction that produces the A partial.
            for pi, ch in enumerate(range(eqw, NCH, 2)):
                gth = gtp.tile([128, 4096], BF16, name="gthp", tag="gthp")
                if pi == 0:
                    for k in range(4):
                        nc.sync.dma_start(
                            out=gth[:, 512 * k : 512 * (k + 1)],
                            in_=gTh[:, 2048 * ch + 512 * k : 2048 * ch + 512 * (k + 1)],
                        )
                    nc.sync.dma_start(
                        out=gth[:, 2048:4096],
                        in_=gTh[:, 2048 * (ch + 1) : 2048 * (ch + 2)],
                    )
                else:
                    for half in range(2):
                        nc.sync.dma_start(
                            out=gth[:, 2048 * half : 2048 * (half + 1)],
                            in_=gTh[:, 2048 * (ch + half) : 2048 * (ch + half + 1)],
                        )
                nc.sync.dma_start(
                    out=cw_sb[:, 1024 * pi : 1024 * (pi + 1)],
                    in_=cw[:, 1024 * pi : 1024 * (pi + 1)],
                )
                for m in range(MT):
                    ps = psp2.tile([128, 1024], F32)
                    for half in range(2):

                        def rh(k, off, n, _g=gth, _h=half):
                            return _g[:, 2048 * _h + 512 * k + off : 2048 * _h + 512 * k + off + n]

                        emit_group(ps[:, 512 * half : 512 * (half + 1)],
                                   None, rh, None, m, False)
                    e1c = e1p.tile([128, 1024], F32, name="e1c", tag="e1c")
                    nc.scalar.activation(
                        e1c[:],
                        ps[:],
                        AF.Exp,
                        scale=10.0,
                    )
                    scw = scr.tile([128, 1024], F32, tag="scrw")
                    nc.vector.scalar_tensor_tensor(
                        out=scw[:],
                        in0=e1c[:],
                        scalar=1.0,
                        in1=cw_sb[:, 1024 * pi : 1024 * (pi + 1)],
                        op0=ALU.mult,
                        op1=ALU.mult,
                        accum_out=a_slot[m][:, pi : pi + 1],
                    )

            full_tiles = []
            for ch in range(eqw):
                fgth = gtp.tile([128, 2048], BF16, name=f"fgth{ch}", tag="gth")
                fgtl = gtp.tile([128, 2048], BF16, name=f"fgtl{ch}", tag="gtl")
                nc.sync.dma_start(out=fgth[:], in_=gTh[:, 2048 * ch : 2048 * (ch + 1)])
                nc.sync.dma_start(out=fgtl[:], in_=gTl[:, 2048 * ch : 2048 * (ch + 1)])
                full_tiles.append((fgth, fgtl))
            for m in range(MT):
                nc.vector.tensor_reduce(
                    a_slot[m][:, ncheap + eqw : ncheap + eqw + 1],
                    a_slot[m][:, 0:ncheap],
                    mybir.AxisListType.X,
                    ALU.add,
                )
            nc.sync.dma_start(out=cst_sb[:], in_=cst[:])
            nc.sync.dma_start(out=ftl[:], in_=fTl[:])
            nc.sync.dma_start(out=ta_sb[:], in_=ta[:])
            nc.sync.dma_start(out=tvec_sb[:], in_=tvec[:])
            nc.sync.dma_start(out=ident_sb[:], in_=ident[:])

            # Full-precision window chunks (matches + diagonal live here).
            # Chunk 0 is the core's own rows; later window chunks hold the
            # matched-other columns; the last window chunk's correction terms
            # narrow to corr_last cols when the host says the matches fit.
            for ch in range(eqw):
                cs = slice(512 * ch, 512 * (ch + 1))
                gth, gtl = full_tiles[ch]
                for m in range(MT):
                    ps = psp.tile([128, 512], F32)

                    def rh(k, off, n, _g=gth):
                        return _g[:, 512 * k + off : 512 * k + off + n]

                    def rl(k, off, n, _g=gtl):
                        return _g[:, 512 * k + off : 512 * k + off + n]

                    if ch == 0 and woff is not None:
                        corr = (woff[m], 256)
                    elif ch == eqw - 1 and corr_last is not None:
                        corr = (0, corr_last)
                    else:
                        corr = None
                    emit_group(ps[:], brow_sb[:, cs], rh, rl, m, True, corr=corr)
                    e1 = e1p.tile([128, 512], F32, name="e1", tag="e1")
                    e1f = e1[:]
                    nc.scalar.activation(
                        e1f,
                        ps[:],
                        AF.Exp,
                        scale=10.0,
                        accum_out=a_slot[m][:, ncheap + ch : ncheap + ch + 1],
                    )
                    sc = scr.tile([128, 512], F32, tag="scr2")
                    nc.vector.scalar_tensor_tensor(
                        out=sc[:],
                        in0=ta_sb[:, cs],
                        scalar=tvec_sb[:, m : m + 1],
                        in1=e1f,
                        op0=ALU.is_equal,
                        op1=ALU.mult,
                        accum_out=s2_slot[m][:, ch : ch + 1],
                    )
                    sc3 = scr.tile([128, 512], F32, tag="scr3")
                    nc.vector.scalar_tensor_tensor(
                        out=sc3[:],
                        in0=ta_sb[:, cs],
                        scalar=tvec_sb[:, m : m + 1],
                        in1=ps[:],
                        op0=ALU.is_equal,
                        op1=ALU.mult,
                        accum_out=s3_slot[m][:, ch : ch + 1],
                    )
                    if ch == 0:
                        # local row p's own column is chunk-0 column 128m+p,
                        # so the psum diagonal of this [128,128] sub-block is
                        # the self dot-product (plus bias) bit-exactly.
                        sd = scr.tile([128, 128], F32, tag="scrd")
                        nc.vector.scalar_tensor_tensor(
                            out=sd[:],
                            in0=ident_sb[:],
                            scalar=1.0,
                            in1=ps[:, 128 * m : 128 * (m + 1)],
                            op0=ALU.mult,
                            op1=ALU.mult,
                            accum_out=outsb[:, 4 * m + 3 : 4 * m + 4],
                        )

            for m in range(MT):
                nc.vector.tensor_reduce(
                    outsb[:, 4 * m : 4 * m + 1],
                    a_slot[m][:, ncheap : ncheap + eqw + 1],
                    mybir.AxisListType.X,
                    ALU.add,
                )
                nc.vector.tensor_reduce(
                    outsb[:, 4 * m + 1 : 4 * m + 2], s2_slot[m][:], mybir.AxisListType.X, ALU.add
                )
                nc.vector.tensor_reduce(
                    outsb[:, 4 * m + 2 : 4 * m + 3], s3_slot[m][:], mybir.AxisListType.X, ALU.add
                )
            nc.sync.dma_start(out=out[:], in_=outsb[:])
    _split_multi_waits(nc)
    return nc


_nc_by_cfg = {}


def _get_nc(eqw, corr_last, woff):
    key = (eqw, corr_last, woff)
    if key not in _nc_by_cfg:
        _nc_by_cfg[key] = _build_nc(eqw, corr_last, woff)
    return _nc_by_cfg[key]


def _prepare(centers1, features, targets, features_ood, pseudo_target_ood):
    """Host-side O(N log N) prep.

    Rows are globally sorted by class and sharded contiguously, so each
    core's 512 rows cover ~C/8 classes whose other members mostly live in
    the same core. Per core the g columns are permuted to
    [own 512 rows | all other same-class batch cols + own-class centers |
     rest bc cols | ood | pad], which confines every eq-match (and the
    diagonal, at column p for local row p) to the first EQW chunks. Only
    those chunks need the 3-term split and the masked S2/S3 reductions.
    """
    centers1 = np.asarray(centers1, np.float32)
    features = np.asarray(features, np.float32)
    features_ood = np.asarray(features_ood, np.float32)
    targets = np.asarray(targets).astype(np.int64)
    pseudo = np.asarray(pseudo_target_ood).astype(np.int64)

    tac = np.concatenate([targets, np.arange(C), pseudo])
    w_full = np.bincount(tac, minlength=C).astype(np.float64)

    # class-id label per g row (incl. centers/ood), and bias per g row
    lab = np.concatenate([targets, np.arange(C), np.full(BO, C, np.int64),
                          np.full(PAD, -1, np.int64)])
    bias1 = np.full(NPAD, -20.0, np.float64)
    bias1[:N] = -(np.log(w_full[tac]) + 10.0) / 10.0
    b_h = bias1.astype(BFNP)
    b_l = (bias1 - b_h.astype(np.float64)).astype(BFNP)

    g = np.concatenate(
        [features, centers1, features_ood, np.zeros((PAD, D), np.float32)], axis=0
    )
    g_h = g.astype(BFNP)
    g_l = (g - g_h.astype(np.float32)).astype(BFNP)

    row_perm = np.argsort(targets, kind="stable")
    t_sorted = targets[row_perm]

    ident = np.eye(128, dtype=np.float32)
    ones2 = np.ones((2, 128), BFNP)

    # per-core column permutations
    perms = []
    eqw_need = 1
    mm_max = 0
    all_batch = np.arange(B)
    for c in range(NCORES):
        own = row_perm[RPC * c : RPC * (c + 1)]            # sorted by class
        tset = np.zeros(C + 1, bool)
        tset[t_sorted[RPC * c : RPC * (c + 1)]] = True
        in_own = np.zeros(B, bool)
        in_own[own] = True
        match_b = all_batch[tset[targets] & ~in_own]       # other cores' rows, own classes
        match_c = B + np.flatnonzero(tset[:C])             # centers of own classes
        matched = np.concatenate([match_b, match_c])
        rest_mask = np.ones(B + C, bool)
        rest_mask[own] = False
        rest_mask[matched] = False
        rest = np.flatnonzero(rest_mask)
        perm = np.concatenate(
            [own, matched, rest,
             np.arange(B + C, N),                          # ood
             np.arange(N, NPAD)]                           # pad
        )
        assert perm.shape == (NPAD,)
        perms.append(perm)
        eqw_need = max(eqw_need, -(-(RPC + len(matched)) // 512))
        mm_max = max(mm_max, RPC + len(matched))

    eqw = max(eqw_need, 2)  # chunks that must carry matches (expected 2)
    # columns the last window chunk must cover at full precision
    rem = mm_max - 512 * (eqw - 1)
    corr_last = 256 if rem <= 256 else None

    # chunk-0 correction windows per row-tile: row-tile m only matches own
    # columns whose classes occur in its rows — a narrow band around 128*m.
    WOFF = (0, 64, 192, 256)
    woff = WOFF
    for c in range(NCORES):
        tc_ = t_sorted[RPC * c : RPC * (c + 1)]
        for m in range(MT):
            cmin, cmax = tc_[128 * m], tc_[128 * m + 127]
            lo = np.searchsorted(tc_, cmin, side="left")
            hi = np.searchsorted(tc_, cmax, side="right")
            if not (WOFF[m] <= lo and hi <= WOFF[m] + 256):
                woff = None
    if woff is None:
        log_fallback = True  # degenerate class spread: keep full-width corrections

    def tile_T(x):
        # [ncols, 512] -> [128, (ncols/512)*2048] in the SBUF chunk layout:
        # block ch at ch*2048, inner offset 512*k + j  (k = dim-slice, j = col)
        nch = x.shape[0] // 512
        xt = np.ascontiguousarray(x.T)                     # [512, ncols]
        return np.ascontiguousarray(
            xt.reshape(4, 128, nch, 512).transpose(1, 2, 0, 3).reshape(128, nch * 2048)
        )

    in_maps = []
    for c in range(NCORES):
        perm = perms[c]
        gTh_c = tile_T(g_h[perm])
        gTl_c = tile_T(g_l[perm[: eqw * 512]])
        fTh_c = tile_T(g_h[perm[:RPC]])
        fTl_c = tile_T(g_l[perm[:RPC]])
        bh_p, bl_p = b_h[perm], b_l[perm]
        beff_p = (bh_p.astype(np.float32) + bl_p.astype(np.float32)).astype(np.float64)
        cw_row = np.exp(10.0 * beff_p[eqw * 512 :]).astype(np.float32)
        cw_bc = np.ascontiguousarray(np.broadcast_to(cw_row, (128, (NCH - eqw) * 512)))
        brow2 = np.stack([bh_p, bl_p])                     # [2, NPAD]
        cst_c = np.ascontiguousarray(
            np.concatenate([ones2, brow2, brow2[:, :RPC]], axis=1).astype(BFNP)
        )
        ta_p = lab[perm[: eqw * 512]].astype(np.float32)
        ta_bc = np.ascontiguousarray(np.broadcast_to(ta_p, (128, eqw * 512)))
        tvec_c = np.ascontiguousarray(
            t_sorted[RPC * c : RPC * (c + 1)].reshape(MT, 128).T.astype(np.float32)
        )
        in_maps.append(
            {
                "gTh": gTh_c,
                "gTl": gTl_c,
                "fTh": fTh_c,
                "fTl": fTl_c,
                "cst": cst_c,
                "ta": ta_bc,
                "tvec": tvec_c,
                "ident": ident,
                "cw": cw_bc,
            }
        )

    # effective per-class bias as the device psum sees it (fp32 add of pair)
    cls_bias = -(np.log(w_full) + 10.0) / 10.0
    cb_h = cls_bias.astype(BFNP)
    cb_l = (cls_bias - cb_h.astype(np.float64)).astype(BFNP)
    bias_eff_cls = (cb_h.astype(np.float32) + cb_l.astype(np.float32)).astype(np.float64)

    host = {"t_sorted": t_sorted, "w_full": w_full, "bias_eff_cls": bias_eff_cls,
            "eqw": eqw, "corr_last": corr_last, "woff": woff}
    return in_maps, host


def _combine(results, host):
    t_sorted = host["t_sorted"]
    w_full = host["w_full"]
    cnt_batch = np.bincount(t_sorted, minlength=C).astype(np.float64)

    A = np.empty(B)
    S2 = np.empty(B)
    S3 = np.empty(B)
    diag = np.empty(B)
    for c in range(NCORES):
        o = np.asarray(results[c]["out"], np.float64)  # [128, 16]
        for m in range(MT):
            rs = slice(RPC * c + 128 * m, RPC * c + 128 * (m + 1))
            A[rs] = o[:, 4 * m]
            S2[rs] = o[:, 4 * m + 1]
            S3[rs] = o[:, 4 * m + 2]
            diag[rs] = o[:, 4 * m + 3]

    ws = w_full[t_sorted]
    K = cnt_batch[t_sorted]
    ds_ = 1.0 / (ws - 1.0) - 1.0 / ws
    b1s = host["bias_eff_cls"][t_sorted]
    e1s = np.exp(10.0 * diag)
    S = A - e1s + ds_ * ws * (S2 - e1s)
    P = 10.0 * (S3 - K * b1s - diag) - 10.0 * K
    val = P / K - np.log(S)
    return np.float32(-val.mean())


def _run(inputs, trace=False, **kw):
    in_maps, host = _prepare(**inputs)
    nc = _get_nc(host["eqw"], host["corr_last"], host["woff"])
    res = run_bass_kernel_spmd(nc, in_maps, list(range(NCORES)), trace=trace, **kw)
    loss = _combine(res.results, host)
    return loss, res


def kernel(**inputs):
    loss, _ = _run(inputs)
    return loss



# revision 2
# speedup vs baseline: 1.3276x; 1.3276x over previous
"""Trainium2 Bass kernel for the class-balanced supervised-contrastive loss.

Math (reference semantics, shift-invariant form with constant shift 10):
  l_ij = (f_i . g_j) / T,  T = 0.1, g = [features; centers; features_ood]
  E_ij = exp(l_ij - 10)
  S_i  = sum_{j != i} E_ij / (w_j - eq_ij)        (w_j = class count, eq = label match)
  P_i  = sum_{j != i} eq_ij (l_ij - 10)
  loss = -mean_i( P_i / K_i - log S_i ),  K_i = batch count of class t_i

Device per core (rows globally sorted by class, 512 rows/core, columns permuted
so every eq-match lands in the first WIN=1024 cols):
  psum = f . g + bias1_col   (bias1 = (ln(1/w) - 10)/10, so exp(10*psum) = E/w)
  A_i  = sum_j exp(10*psum)            ACT exp accum_out
  S2_i = sum_{win} eq * E1             DVE masked reduce (window only)
  S3_i = sum_{win} eq * psum           DVE masked reduce (window only)
  diag = psum_ii                       self column, for exclusion terms

All matmuls run as fp8e4 DoubleRow (2 contraction rows per PE pass):
  - main terms: f8h . g8h over K=512 as 2 DR passes of K_eff=256
  - window adds f8l.g8h + f8h.g8l correction terms (dot err ~2.5e-4) plus an
    exact bf16 (hi,lo) K=2 bias matmul
  - cheap (non-window) cols get their bias as a 3-row fp8 DR matmul
    (bh,bm,bl residual cascade, exponent err ~1e-2 -> A err ~1e-4), which is
    also the bank-opening start=True instruction for psum zeroing
Column space per (m-tile) is processed in [128,2048] psum groups (4 banks,
double buffered), each consumed by one wide ACT exp with accum_out.
"""

import ml_dtypes
import numpy as np

import concourse.bass as bass
import concourse.mybir as mybir
import concourse.tile as tile
from concourse.bass_utils import run_bass_kernel_spmd

NCORES = 8
C, TEMP = 1000, 0.1
B, BO, D = 4096, 4096, 512
N = B + C + BO              # 9192
NPAD = 9216                 # 18 * 512
PAD = NPAD - N
NCH = NPAD // 512           # 18 column chunks
RPC = B // NCORES           # 512 rows per core
MT = RPC // 128             # 4 row tiles per core

F32 = mybir.dt.float32
BF16 = mybir.dt.bfloat16
FP8 = mybir.dt.float8e4
DR = mybir.MatmulPerfMode.DoubleRow
ALU = mybir.AluOpType
AF = mybir.ActivationFunctionType
BFNP = ml_dtypes.bfloat16
FP8NP = ml_dtypes.float8_e4m3

# This walrus build accepts only one sync-wait command per engine instruction.
# Move surplus waits onto standalone EventSemaphore instructions just before
# the affected instruction (same engine, so blocking semantics are identical).
_SPLIT_SKIP = ("InstEventSemaphore",)


def _split_multi_waits(nc):
    n = 0
    for f in nc.m.functions:
        for bb in f.blocks:
            new = []
            for ins in bb.instructions:
                si = ins.sync_info
                if (
                    si is not None
                    and si.on_wait
                    and len(si.on_wait) > 1
                    and type(ins).__name__ not in _SPLIT_SKIP
                ):
                    waits = list(si.on_wait)
                    for w in waits[:-1]:
                        es = mybir.InstEventSemaphore(
                            name=f"wsplit_{n}",
                            engine=ins.engine,
                            sync_info=mybir.SyncInfo(on_wait=[w], on_update=[]),
                        )
                        n += 1
                        new.append(es)
                    ins.sync_info = mybir.SyncInfo(
                        on_wait=[waits[-1]], on_update=list(si.on_update)
                    )
                new.append(ins)
            bb.instructions = new
    return n


def _build_nc(wch=2):
    """wch = number of 512-col window chunks holding all eq-matches."""
    cch = NCH - wch                     # cheap chunks
    win = 512 * wch
    # cheap chunks packed into psum groups of <=4 chunks
    groups = []
    ch = wch
    while ch < NCH:
        g = list(range(ch, min(ch + 4, NCH)))
        groups.append(g)
        ch += len(g)
    ngrp = len(groups) + 1              # + window group
    nc = bass.Bass()

    # DR layouts: contraction row r = 256*khat + 128*i + p  (pair i, partition p)
    gT8 = nc.declare_dram_parameter("gT8", [128, NCH * 2048], FP8, isOutput=False)
    gT8l = nc.declare_dram_parameter("gT8l", [128, wch * 2048], FP8, isOutput=False)
    fT8 = nc.declare_dram_parameter("fT8", [128, MT * 512], FP8, isOutput=False)
    fT8l = nc.declare_dram_parameter("fT8l", [128, MT * 512], FP8, isOutput=False)
    ones8 = nc.declare_dram_parameter("ones8", [2, 256], FP8, isOutput=False)
    b8 = nc.declare_dram_parameter("b8", [2, cch * 1024], FP8, isOutput=False)
    ones2 = nc.declare_dram_parameter("ones2", [2, 128], BF16, isOutput=False)
    bw = nc.declare_dram_parameter("bw", [2, win], BF16, isOutput=False)
    ta = nc.declare_dram_parameter("ta", [128, win], F32, isOutput=False)
    tvec = nc.declare_dram_parameter("tvec", [128, MT], F32, isOutput=False)
    ident = nc.declare_dram_parameter("ident", [128, 128], F32, isOutput=False)
    out = nc.declare_dram_parameter("out", [128, 4 * MT], F32, isOutput=True)

    with tile.TileContext(nc) as tc:
        with (
            tc.tile_pool(name="const", bufs=1) as const,
            tc.tile_pool(name="stats", bufs=1) as stats,
            tc.tile_pool(name="gt", bufs=8) as gtp,
            tc.tile_pool(name="e1c", bufs=3) as e1cp,
            tc.tile_pool(name="e1w", bufs=2) as e1wp,
            tc.tile_pool(name="psum", bufs=2, space="PSUM") as psp,
        ):
            ft = const.tile([128, MT, 2, 2, 128], FP8)
            ftl = const.tile([128, MT, 2, 2, 128], FP8)
            ones8_sb = const.tile([2, 2, 128], FP8)
            b8_sb = const.tile([2, cch, 2, 512], FP8)
            ones2_sb = const.tile([2, 128], BF16)
            bw_sb = const.tile([2, win], BF16)
            ta_sb = const.tile([128, win], F32)
            tvec_sb = const.tile([128, MT], F32)
            ident_sb = const.tile([128, 128], F32)
            gl = const.tile([128, wch, 2, 2, 512], FP8)

            nc.sync.dma_start(out=ft[:], in_=fT8[:])
            nc.sync.dma_start(out=ones8_sb[:], in_=ones8[:])
            nc.sync.dma_start(out=b8_sb[:], in_=b8[:])

            outsb = stats.tile([128, 4 * MT], F32)
            # per m: ngrp A-partials | wch S2 parts | wch S3 parts
            acc = [stats.tile([128, ngrp + 2 * wch], F32, name=f"acc{m}") for m in range(MT)]

            # -- cheap group 0 first (PE can start on 4 chunk DMAs + ft) -----
            def cheap_group(gi, g):
                gts = []
                for ch in g:
                    gt = gtp.tile([128, 2, 2, 512], FP8, name=f"g{ch}", tag="gt")
                    nc.sync.dma_start(
                        out=gt[:], in_=gT8[:, 2048 * ch : 2048 * (ch + 1)]
                    )
                    gts.append(gt)
                for m in range(MT):
                    ps = psp.tile([128, 2048], F32)
                    for ci, ch in enumerate(g):
                        cs = slice(512 * ci, 512 * (ci + 1))
                        nc.tensor.matmul(
                            ps[:, cs], ones8_sb[:], b8_sb[:, ch - wch],
                            start=True, stop=False, perf_mode=DR,
                        )
                        for k in range(2):
                            nc.tensor.matmul(
                                ps[:, cs], ft[:, m, k], gts[ci][:, k],
                                start=False, stop=(k == 1), perf_mode=DR,
                            )
                    e1c = e1cp.tile([128, 512 * len(g)], BF16, tag="e1c")
                    nc.scalar.activation(
                        e1c[:], ps[:, : 512 * len(g)], AF.Exp, scale=10.0,
                        accum_out=acc[m][:, 1 + gi : 2 + gi],
                    )

            cheap_group(0, groups[0])

            # window DMAs land while group 0 computes
            nc.sync.dma_start(out=ftl[:], in_=fT8l[:])
            nc.sync.dma_start(out=ones2_sb[:], in_=ones2[:])
            nc.sync.dma_start(out=bw_sb[:], in_=bw[:])
            nc.sync.dma_start(out=ta_sb[:], in_=ta[:])
            nc.sync.dma_start(out=tvec_sb[:], in_=tvec[:])
            nc.sync.dma_start(out=ident_sb[:], in_=ident[:])
            for ch in range(wch):
                nc.sync.dma_start(
                    out=gl[:, ch], in_=gT8l[:, 2048 * ch : 2048 * (ch + 1)]
                )
                gt = gtp.tile([128, 2, 2, 512], FP8, name=f"gw{ch}", tag="gtw")
                nc.sync.dma_start(out=gt[:], in_=gT8[:, 2048 * ch : 2048 * (ch + 1)])
                if ch == 0:
                    gw = [gt]
                else:
                    gw.append(gt)

            # -- window groups: full-precision matmuls + masked reductions ---
            for m in range(MT):
                ps = psp.tile([128, 2048], F32)
                for ch in range(wch):
                    cs = slice(512 * ch, 512 * (ch + 1))
                    nc.tensor.matmul(
                        ps[:, cs], ones2_sb[:], bw_sb[:, cs],
                        start=True, stop=False,
                    )
                    for lhs, rhs in ((ft, gw[ch]), (ftl, gw[ch]), (ft, gl)):
                        rhs_is_gl = rhs is gl
                        for k in range(2):
                            nc.tensor.matmul(
                                ps[:, cs],
                                lhs[:, m, k],
                                gl[:, ch, k] if rhs_is_gl else rhs[:, k],
                                start=False,
                                stop=(rhs_is_gl and k == 1),
                                perf_mode=DR,
                            )
                e1w = e1wp.tile([128, win], F32, tag="e1w")
                nc.scalar.activation(
                    e1w[:], ps[:, :win], AF.Exp, scale=10.0,
                    accum_out=acc[m][:, 0:1],
                )
                # psum readers first so the buffer frees quickly
                sd = e1wp.tile([128, 128], F32, tag="scrd")
                nc.vector.scalar_tensor_tensor(
                    out=sd[:], in0=ident_sb[:], scalar=1.0,
                    in1=ps[:, 128 * m : 128 * (m + 1)],
                    op0=ALU.mult, op1=ALU.mult,
                    accum_out=outsb[:, 4 * m + 3 : 4 * m + 4],
                )
                for ch in range(wch):
                    cs = slice(512 * ch, 512 * (ch + 1))
                    sc3 = e1wp.tile([128, 512], F32, tag="scr3")
                    nc.vector.scalar_tensor_tensor(
                        out=sc3[:], in0=ta_sb[:, cs], scalar=tvec_sb[:, m : m + 1],
                        in1=ps[:, cs], op0=ALU.is_equal, op1=ALU.mult,
                        accum_out=acc[m][:, ngrp + wch + ch : ngrp + wch + ch + 1],
                    )
                for ch in range(wch):
                    cs = slice(512 * ch, 512 * (ch + 1))
                    sc = e1wp.tile([128, 512], F32, tag="scr2")
                    nc.vector.scalar_tensor_tensor(
                        out=sc[:], in0=ta_sb[:, cs], scalar=tvec_sb[:, m : m + 1],
                        in1=e1w[:, cs], op0=ALU.is_equal, op1=ALU.mult,
                        accum_out=acc[m][:, ngrp + ch : ngrp + ch + 1],
                    )

            # -- remaining cheap groups -------------------------------------
            for gi, g in enumerate(groups[1:], start=1):
                cheap_group(gi, g)

            for m in range(MT):
                nc.vector.tensor_reduce(
                    outsb[:, 4 * m : 4 * m + 1], acc[m][:, 0:ngrp],
                    mybir.AxisListType.X, ALU.add,
                )
                nc.vector.tensor_reduce(
                    outsb[:, 4 * m + 1 : 4 * m + 2], acc[m][:, ngrp : ngrp + wch],
                    mybir.AxisListType.X, ALU.add,
                )
                nc.vector.tensor_reduce(
                    outsb[:, 4 * m + 2 : 4 * m + 3],
                    acc[m][:, ngrp + wch : ngrp + 2 * wch],
                    mybir.AxisListType.X, ALU.add,
                )
            nc.sync.dma_start(out=out[:], in_=outsb[:])
    _split_multi_waits(nc)
    return nc


_nc_by_cfg = {}


def _get_nc(wch):
    if wch not in _nc_by_cfg:
        _nc_by_cfg[wch] = _build_nc(wch)
    return _nc_by_cfg[wch]


def _fp8_cascade(x, n):
    """Split x into n fp8 rows summing (in f32) to ~x."""
    rows = []
    rem = np.asarray(x, np.float64).copy()
    for _ in range(n):
        h = rem.astype(FP8NP)
        rows.append(h)
        rem = rem - h.astype(np.float64)
    return rows


def _dr_tile(x):
    """[ncols, 512] fp8 -> [128, ncols/512 * 2048] in the DR chunk layout:
    [p, ch*2048 + (khat*2 + i)*512 + j] = x[512*ch + j, 256*khat + 128*i + p]."""
    nch = x.shape[0] // 512
    xt = np.ascontiguousarray(x.T)                  # [512, ncols]
    return np.ascontiguousarray(
        xt.reshape(2, 2, 128, nch, 512).transpose(2, 3, 0, 1, 4).reshape(128, -1)
    )


def _dr_tile_f(x):
    """[512 rows, 512 dims] fp8 -> [128, MT*512] stationary layout:
    [p, ((m*2 + khat)*2 + i)*128 + q] = x[128*m + q, 256*khat + 128*i + p]."""
    xt = np.ascontiguousarray(x.T)                  # [512 dims, 512 rows]
    return np.ascontiguousarray(
        xt.reshape(2, 2, 128, MT, 128).transpose(2, 3, 0, 1, 4).reshape(128, -1)
    )


def _prepare(centers1, features, targets, features_ood, pseudo_target_ood):
    """Host-side prep: sort rows by class, shard contiguously, and per core
    permute the g columns to [own 512 | matched | rest | ood | pad] so all
    eq-matches (and the diagonal, at window column 128m+p) land in the first
    WIN columns."""
    centers1 = np.asarray(centers1, np.float32)
    features = np.asarray(features, np.float32)
    features_ood = np.asarray(features_ood, np.float32)
    targets = np.asarray(targets).astype(np.int64)
    pseudo = np.asarray(pseudo_target_ood).astype(np.int64)

    tac = np.concatenate([targets, np.arange(C), pseudo])
    w_full = np.bincount(tac, minlength=C).astype(np.float64)

    # class-id label per g row (incl. centers/ood), and bias per g row
    lab = np.concatenate([targets, np.arange(C), np.full(BO, C, np.int64),
                          np.full(PAD, -1, np.int64)])
    bias1 = np.full(NPAD, -20.0, np.float64)
    bias1[:N] = -(np.log(w_full[tac]) + 10.0) / 10.0

    g = np.concatenate(
        [features, centers1, features_ood, np.zeros((PAD, D), np.float32)], axis=0
    )
    g8h = g.astype(FP8NP)
    g8l = (g - g8h.astype(np.float32)).astype(FP8NP)

    row_perm = np.argsort(targets, kind="stable")
    t_sorted = targets[row_perm]

    # per-core column permutations
    perms = []
    win_need = 1
    all_batch = np.arange(B)
    for c in range(NCORES):
        own = row_perm[RPC * c : RPC * (c + 1)]            # sorted by class
        tset = np.zeros(C + 1, bool)
        tset[t_sorted[RPC * c : RPC * (c + 1)]] = True
        in_own = np.zeros(B, bool)
        in_own[own] = True
        match_b = all_batch[tset[targets] & ~in_own]       # other cores' rows, own classes
        match_c = B + np.flatnonzero(tset[:C])             # centers of own classes
        matched = np.concatenate([match_b, match_c])
        rest_mask = np.ones(B + C, bool)
        rest_mask[own] = False
        rest_mask[matched] = False
        rest = np.flatnonzero(rest_mask)
        perm = np.concatenate(
            [own, matched, rest,
             np.arange(B + C, N),                          # ood
             np.arange(N, NPAD)]                           # pad
        )
        assert perm.shape == (NPAD,)
        perms.append(perm)
        win_need = max(win_need, RPC + len(matched))

    wch = max(2, -(-win_need // 512))
    win = 512 * wch
    cch = NCH - wch

    # window bias rows (bf16 hi+lo) and the device-effective per-class value
    cls_bias = -(np.log(w_full) + 10.0) / 10.0
    cb_h = cls_bias.astype(BFNP)
    cb_l = (cls_bias - cb_h.astype(np.float64)).astype(BFNP)
    bias_eff_cls = (cb_h.astype(np.float32) + cb_l.astype(np.float32)).astype(np.float64)

    bh_all = bias1.astype(BFNP)
    bl_all = (bias1 - bh_all.astype(np.float64)).astype(BFNP)
    b8_rows = _fp8_cascade(bias1, 3)                       # bh, bm, bl fp8

    ones8_host = np.zeros((2, 2, 128), np.float32)
    ones8_host[0, 0] = 1.0
    ones8_host[0, 1] = 1.0
    ones8_host[1, 0] = 1.0
    ones2_host = np.ones((2, 128), np.float32)
    ident = np.eye(128, dtype=np.float32)

    in_maps = []
    for c in range(NCORES):
        perm = perms[c]
        own = perm[:RPC]
        f8h = g8h[own]                                     # [512, 512] fp8
        f8l = g8l[own]
        # cheap bias rows: [p, cch_idx, i, j] with (0,0)=bh (0,1)=bm (1,0)=bl
        b8c = np.zeros((2, cch, 2, 512), FP8NP)
        pc = perm[win:].reshape(cch, 512)
        b8c[0, :, 0] = b8_rows[0][pc]
        b8c[0, :, 1] = b8_rows[1][pc]
        b8c[1, :, 0] = b8_rows[2][pc]
        bw_c = np.stack([bh_all[perm[:win]], bl_all[perm[:win]]])
        ta_p = lab[perm[:win]].astype(np.float32)
        in_maps.append(
            {
                "gT8": _dr_tile(g8h[perm]),
                "gT8l": _dr_tile(g8l[perm[:win]]),
                "fT8": _dr_tile_f(f8h),
                "fT8l": _dr_tile_f(f8l),
                "ones8": np.ascontiguousarray(ones8_host.reshape(2, 256).astype(FP8NP)),
                "b8": np.ascontiguousarray(b8c.reshape(2, cch * 1024)),
                "ones2": np.ascontiguousarray(ones2_host.astype(BFNP)),
                "bw": np.ascontiguousarray(bw_c.astype(BFNP)),
                "ta": np.ascontiguousarray(np.broadcast_to(ta_p, (128, win))),
                "tvec": np.ascontiguousarray(
                    t_sorted[RPC * c : RPC * (c + 1)].reshape(MT, 128).T.astype(np.float32)
                ),
                "ident": ident,
            }
        )

    host = {"t_sorted": t_sorted, "w_full": w_full, "bias_eff_cls": bias_eff_cls,
            "wch": wch}
    return in_maps, host


def _combine(results, host):
    t_sorted = host["t_sorted"]
    w_full = host["w_full"]
    cnt_batch = np.bincount(t_sorted, minlength=C).astype(np.float64)

    A = np.empty(B)
    S2 = np.empty(B)
    S3 = np.empty(B)
    diag = np.empty(B)
    for c in range(NCORES):
        o = np.asarray(results[c]["out"], np.float64)  # [128, 16]
        for m in range(MT):
            rs = slice(RPC * c + 128 * m, RPC * c + 128 * (m + 1))
            A[rs] = o[:, 4 * m]
            S2[rs] = o[:, 4 * m + 1]
            S3[rs] = o[:, 4 * m + 2]
            diag[rs] = o[:, 4 * m + 3]

    ws = w_full[t_sorted]
    K = cnt_batch[t_sorted]
    ds_ = 1.0 / (ws - 1.0) - 1.0 / ws
    b1s = host["bias_eff_cls"][t_sorted]
    e1s = np.exp(10.0 * diag)
    S = A - e1s + ds_ * ws * (S2 - e1s)
    P = 10.0 * (S3 - K * b1s - diag) - 10.0 * K
    val = P / K - np.log(S)
    return np.float32(-val.mean())


def _run(inputs, trace=False, **kw):
    in_maps, host = _prepare(**inputs)
    nc = _get_nc(host["wch"])
    res = run_bass_kernel_spmd(nc, in_maps, list(range(NCORES)), trace=trace, **kw)
    loss = _combine(res.results, host)
    return loss, res


def kernel(**inputs):
    loss, _ = _run(inputs)
    return loss


# revision 7
# speedup vs baseline: 1.4552x; 1.0961x over previous
"""Trainium2 Bass kernel for the class-balanced supervised-contrastive loss.

Math (reference semantics, shift-invariant form with constant shift 10):
  l_ij = (f_i . g_j) / T,  T = 0.1, g = [features; centers; features_ood]
  E_ij = exp(l_ij - 10)
  S_i  = sum_{j != i} E_ij / (w_j - eq_ij)        (w_j = class count, eq = label match)
  P_i  = sum_{j != i} eq_ij (l_ij - 10)
  loss = -mean_i( P_i / K_i - log S_i ),  K_i = batch count of class t_i

Device per core (rows globally sorted by class, 512 rows/core, columns permuted
so every eq-match lands in the first WIN=1024 cols):
  psum = f . g + bias1_col   (bias1 = (ln(1/w) - 10)/10, so exp(10*psum) = E/w)
  A_i  = sum_j exp(10*psum)            ACT exp accum_out
  S2_i = sum_{win} eq * E1             DVE masked reduce (window only)
  S3_i = sum_{win} eq * psum           DVE masked reduce (window only)
  diag = psum_ii                       self column, for exclusion terms

All matmuls run as fp8e4 DoubleRow (2 contraction rows per PE pass):
  - main terms: f8h . g8h over K=512 as 2 DR passes of K_eff=256
  - window adds f8l.g8h + f8h.g8l correction terms (dot err ~2.5e-4) plus an
    exact bf16 (hi,lo) K=2 bias matmul
  - cheap (non-window) cols get their bias as a 3-row fp8 DR matmul
    (bh,bm,bl residual cascade, exponent err ~1e-2 -> A err ~1e-4), which is
    also the bank-opening start=True instruction for psum zeroing
Column space per (m-tile) is processed in [128,2048] psum groups (4 banks,
double buffered), each consumed by one wide ACT exp with accum_out.
"""

import ml_dtypes
import numpy as np

import concourse.bass as bass
import concourse.mybir as mybir
import concourse.tile as tile
from concourse.bass_utils import run_bass_kernel_spmd

NCORES = 8
C, TEMP = 1000, 0.1
B, BO, D = 4096, 4096, 512
N = B + C + BO              # 9192
NPAD = 9216                 # 18 * 512
PAD = NPAD - N
NCH = NPAD // 512           # 18 column chunks
RPC = B // NCORES           # 512 rows per core
MT = RPC // 128             # 4 row tiles per core

F32 = mybir.dt.float32
BF16 = mybir.dt.bfloat16
FP8 = mybir.dt.float8e4
DR = mybir.MatmulPerfMode.DoubleRow
ALU = mybir.AluOpType
AF = mybir.ActivationFunctionType
BFNP = ml_dtypes.bfloat16
FP8NP = ml_dtypes.float8_e4m3

# This walrus build accepts only one sync-wait command per engine instruction.
# Move surplus waits onto standalone EventSemaphore instructions just before
# the affected instruction (same engine, so blocking semantics are identical).
_SPLIT_SKIP = ("InstEventSemaphore",)


def _split_multi_waits(nc):
    n = 0
    for f in nc.m.functions:
        for bb in f.blocks:
            new = []
            for ins in bb.instructions:
                si = ins.sync_info
                if (
                    si is not None
                    and si.on_wait
                    and len(si.on_wait) > 1
                    and type(ins).__name__ not in _SPLIT_SKIP
                ):
                    waits = list(si.on_wait)
                    for w in waits[:-1]:
                        es = mybir.InstEventSemaphore(
                            name=f"wsplit_{n}",
                            engine=ins.engine,
                            sync_info=mybir.SyncInfo(on_wait=[w], on_update=[]),
                        )
                        n += 1
                        new.append(es)
                    ins.sync_info = mybir.SyncInfo(
                        on_wait=[waits[-1]], on_update=list(si.on_update)
                    )
                new.append(ins)
            bb.instructions = new
    return n


def _build_nc(wch=2):
    """wch = number of 512-col window chunks holding all eq-matches."""
    cch = NCH - wch                     # cheap chunks
    win = 512 * wch
    # cheap chunks packed into psum groups of <=4 chunks
    groups = []
    ch = wch
    while ch < NCH:
        g = list(range(ch, min(ch + 4, NCH)))
        groups.append(g)
        ch += len(g)
    ngrp = len(groups) + 1              # + window group
    nc = bass.Bass()

    # DR layouts: contraction row r = 256*khat + 128*i + p  (pair i, partition p)
    gT8 = nc.declare_dram_parameter("gT8", [128, NCH * 2048], FP8, isOutput=False)
    gT8l = nc.declare_dram_parameter("gT8l", [128, wch * 2048], FP8, isOutput=False)
    fT8 = nc.declare_dram_parameter("fT8", [128, MT * 512], FP8, isOutput=False)
    fT8l = nc.declare_dram_parameter("fT8l", [128, MT * 512], FP8, isOutput=False)
    warm = nc.declare_dram_parameter("warm", [2, 1024], FP8, isOutput=False)
    ones8 = nc.declare_dram_parameter("ones8", [2, 256], FP8, isOutput=False)
    b8 = nc.declare_dram_parameter("b8", [2, cch * 1024], FP8, isOutput=False)
    ones2 = nc.declare_dram_parameter("ones2", [2, 128], BF16, isOutput=False)
    bw = nc.declare_dram_parameter("bw", [2, win], BF16, isOutput=False)
    ta = nc.declare_dram_parameter("ta", [128, win], F32, isOutput=False)
    tvec = nc.declare_dram_parameter("tvec", [128, MT], F32, isOutput=False)
    ident = nc.declare_dram_parameter("ident", [128, 128], F32, isOutput=False)
    out = nc.declare_dram_parameter("out", [128, 4 * MT], F32, isOutput=True)

    with tile.TileContext(nc) as tc:
        with (
            tc.tile_pool(name="const", bufs=1) as const,
            tc.tile_pool(name="stats", bufs=1) as stats,
            tc.tile_pool(name="gt", bufs=8) as gtp,
            tc.tile_pool(name="e1c", bufs=3) as e1cp,
            tc.tile_pool(name="e1w", bufs=2) as e1wp,
            tc.tile_pool(name="psum", bufs=2, space="PSUM") as psp,
        ):
            ft = const.tile([128, MT, 2, 2, 128], FP8)
            ftl = const.tile([128, MT, 2, 2, 128], FP8)
            warm_sb = const.tile([2, 2, 512], FP8)
            warm_o = const.tile([2, 512], F32)
            ones8_sb = const.tile([2, 2, 128], FP8)
            b8_sb = const.tile([2, cch, 2, 512], FP8)
            ones2_sb = const.tile([2, 128], BF16)
            bw_sb = const.tile([2, win], BF16)
            ta_sb = const.tile([128, win], F32)
            tvec_sb = const.tile([128, MT], F32)
            ident_sb = const.tile([128, 128], F32)
            gl = const.tile([128, wch, 2, 2, 512], FP8)

            nc.sync.dma_start(out=warm_sb[:], in_=warm[:])
            nc.sync.dma_start(out=ones8_sb[:], in_=ones8[:])
            nc.sync.dma_start(out=ft[:], in_=fT8[:])
            # Exp table preload off the critical path
            nc.scalar.activation(warm_o[:], warm_sb[:, 0], AF.Exp, scale=1.0)

            outsb = stats.tile([128, 4 * MT], F32)
            # per m: ngrp A-partials | wch S2 parts | wch S3 parts
            acc = [stats.tile([128, ngrp + 2 * wch], F32, name=f"acc{m}") for m in range(MT)]

            group_tiles = {}

            def load_group(gi, g):
                nc.sync.dma_start(
                    out=b8_sb[:, g[0] - wch : g[-1] + 1 - wch],
                    in_=b8[:, 1024 * (g[0] - wch) : 1024 * (g[-1] + 1 - wch)],
                )
                gts = []
                for ch in g:
                    gt = gtp.tile([128, 2, 2, 512], FP8, name=f"g{ch}", tag="gt")
                    nc.sync.dma_start(
                        out=gt[:], in_=gT8[:, 2048 * ch : 2048 * (ch + 1)]
                    )
                    gts.append(gt)
                group_tiles[gi] = gts

            def cheap_unit(gi, g, m, warmups=0):
                gts = group_tiles[gi]
                ps = psp.tile([128, 2048], F32)
                # PE warmup/p-state ramp spins while the first DMAs land
                for _ in range(warmups):
                    nc.tensor.matmul(
                        ps[:, 0:512], warm_sb[:, :, :128], warm_sb[:],
                        start=True, stop=True, perf_mode=DR, skip_group_check=True,
                    )
                for ci, ch in enumerate(g):
                    cs = slice(512 * ci, 512 * (ci + 1))
                    nc.tensor.matmul(
                        ps[:, cs], ones8_sb[:], b8_sb[:, ch - wch],
                        start=True, stop=False, perf_mode=DR,
                    )
                    for k in range(2):
                        nc.tensor.matmul(
                            ps[:, cs], ft[:, m, k], gts[ci][:, k],
                            start=False, stop=(k == 1), perf_mode=DR,
                        )
                e1c = e1cp.tile([128, 512 * len(g)], BF16, tag="e1c")
                nc.scalar.activation(
                    e1c[:], ps[:, : 512 * len(g)], AF.Exp, scale=10.0,
                    accum_out=acc[m][:, 1 + gi : 2 + gi],
                )

            load_group(0, groups[0])
            cheap_unit(0, groups[0], 0, warmups=12)
            for m in range(1, MT):
                cheap_unit(0, groups[0], m)

            # window DMAs (idle Pool queue) land while group 0 computes
            nc.gpsimd.dma_start(out=ftl[:], in_=fT8l[:])
            nc.gpsimd.dma_start(out=ones2_sb[:], in_=ones2[:])
            nc.gpsimd.dma_start(out=bw_sb[:], in_=bw[:])
            nc.gpsimd.dma_start(out=ta_sb[:], in_=ta[:])
            nc.gpsimd.dma_start(out=tvec_sb[:], in_=tvec[:])
            nc.gpsimd.dma_start(out=ident_sb[:], in_=ident[:])
            gw = []
            for ch in range(wch):
                nc.gpsimd.dma_start(
                    out=gl[:, ch], in_=gT8l[:, 2048 * ch : 2048 * (ch + 1)]
                )
                gt = gtp.tile([128, 2, 2, 512], FP8, name=f"gw{ch}", tag="gtw")
                nc.gpsimd.dma_start(out=gt[:], in_=gT8[:, 2048 * ch : 2048 * (ch + 1)])
                gw.append(gt)

            for m in range(MT):
                ps = psp.tile([128, 2048], F32)
                for ch in range(wch):
                    cs = slice(512 * ch, 512 * (ch + 1))
                    nc.tensor.matmul(
                        ps[:, cs], ones2_sb[:], bw_sb[:, cs],
                        start=True, stop=False,
                    )
                    terms = (
                        (ft, (gw[ch][:, 0], gw[ch][:, 1])),
                        (ftl, (gw[ch][:, 0], gw[ch][:, 1])),
                        (ft, (gl[:, ch, 0], gl[:, ch, 1])),
                    )
                    for ti, (lhs, rhss) in enumerate(terms):
                        for k in range(2):
                            nc.tensor.matmul(
                                ps[:, cs], lhs[:, m, k], rhss[k],
                                start=False,
                                stop=(ti == 2 and k == 1),
                                perf_mode=DR,
                            )
                e1w = e1wp.tile([128, win], F32, tag="e1w")
                nc.scalar.activation(
                    e1w[:], ps[:, :win], AF.Exp, scale=10.0,
                    accum_out=acc[m][:, 0:1],
                )
                # psum readers run concurrently with the exp, freeing ps fast
                sd = e1wp.tile([128, 128], F32, tag="scrd")
                nc.vector.scalar_tensor_tensor(
                    out=sd[:], in0=ident_sb[:], scalar=1.0,
                    in1=ps[:, 128 * m : 128 * (m + 1)],
                    op0=ALU.mult, op1=ALU.mult,
                    accum_out=outsb[:, 4 * m + 3 : 4 * m + 4],
                )
                for ch in range(wch):
                    cs = slice(512 * ch, 512 * (ch + 1))
                    sc3 = e1wp.tile([128, 512], F32, tag="scr3")
                    nc.vector.scalar_tensor_tensor(
                        out=sc3[:], in0=ta_sb[:, cs], scalar=tvec_sb[:, m : m + 1],
                        in1=ps[:, cs], op0=ALU.is_equal, op1=ALU.mult,
                        accum_out=acc[m][:, ngrp + wch + ch : ngrp + wch + ch + 1],
                    )
                for ch in range(wch):
                    cs = slice(512 * ch, 512 * (ch + 1))
                    sc = e1wp.tile([128, 512], F32, tag="scr2")
                    nc.vector.scalar_tensor_tensor(
                        out=sc[:], in0=ta_sb[:, cs], scalar=tvec_sb[:, m : m + 1],
                        in1=e1w[:, cs], op0=ALU.is_equal, op1=ALU.mult,
                        accum_out=acc[m][:, ngrp + ch : ngrp + ch + 1],
                    )
                # interleave one cheap unit after each window group so ACT
                # always has a 2048-wide exp queued while DVE drains the window
                if m < MT:
                    if m == 0:
                        load_group(1, groups[1])
                    cheap_unit(1, groups[1], m)

            # -- remaining cheap groups -------------------------------------
            for gi, g in enumerate(groups[2:], start=2):
                load_group(gi, g)
                for m in range(MT):
                    cheap_unit(gi, g, m)

            for m in range(MT):
                nc.vector.tensor_reduce(
                    outsb[:, 4 * m : 4 * m + 1], acc[m][:, 0:ngrp],
                    mybir.AxisListType.X, ALU.add,
                )
                nc.vector.tensor_reduce(
                    outsb[:, 4 * m + 1 : 4 * m + 2], acc[m][:, ngrp : ngrp + wch],
                    mybir.AxisListType.X, ALU.add,
                )
                nc.vector.tensor_reduce(
                    outsb[:, 4 * m + 2 : 4 * m + 3],
                    acc[m][:, ngrp + wch : ngrp + 2 * wch],
                    mybir.AxisListType.X, ALU.add,
                )
            nc.sync.dma_start(out=out[:], in_=outsb[:])
    _split_multi_waits(nc)
    return nc


_nc_by_cfg = {}


def _get_nc(wch):
    if wch not in _nc_by_cfg:
        _nc_by_cfg[wch] = _build_nc(wch)
    return _nc_by_cfg[wch]


def _fp8_cascade(x, n):
    """Split x into n fp8 rows summing (in f32) to ~x."""
    rows = []
    rem = np.asarray(x, np.float64).copy()
    for _ in range(n):
        h = rem.astype(FP8NP)
        rows.append(h)
        rem = rem - h.astype(np.float64)
    return rows


def _dr_tile(x):
    """[ncols, 512] fp8 -> [128, ncols/512 * 2048] in the DR chunk layout:
    [p, ch*2048 + (khat*2 + i)*512 + j] = x[512*ch + j, 256*khat + 128*i + p]."""
    nch = x.shape[0] // 512
    xt = np.ascontiguousarray(x.T)                  # [512, ncols]
    return np.ascontiguousarray(
        xt.reshape(2, 2, 128, nch, 512).transpose(2, 3, 0, 1, 4).reshape(128, -1)
    )


def _dr_tile_f(x):
    """[512 rows, 512 dims] fp8 -> [128, MT*512] stationary layout:
    [p, ((m*2 + khat)*2 + i)*128 + q] = x[128*m + q, 256*khat + 128*i + p]."""
    xt = np.ascontiguousarray(x.T)                  # [512 dims, 512 rows]
    return np.ascontiguousarray(
        xt.reshape(2, 2, 128, MT, 128).transpose(2, 3, 0, 1, 4).reshape(128, -1)
    )


def _prepare(centers1, features, targets, features_ood, pseudo_target_ood):
    """Host-side prep: sort rows by class, shard contiguously, and per core
    permute the g columns to [own 512 | matched | rest | ood | pad] so all
    eq-matches (and the diagonal, at window column 128m+p) land in the first
    WIN columns."""
    centers1 = np.asarray(centers1, np.float32)
    features = np.asarray(features, np.float32)
    features_ood = np.asarray(features_ood, np.float32)
    targets = np.asarray(targets).astype(np.int64)
    pseudo = np.asarray(pseudo_target_ood).astype(np.int64)

    tac = np.concatenate([targets, np.arange(C), pseudo])
    w_full = np.bincount(tac, minlength=C).astype(np.float64)

    # class-id label per g row (incl. centers/ood), and bias per g row
    lab = np.concatenate([targets, np.arange(C), np.full(BO, C, np.int64),
                          np.full(PAD, -1, np.int64)])
    bias1 = np.full(NPAD, -20.0, np.float64)
    bias1[:N] = -(np.log(w_full[tac]) + 10.0) / 10.0

    g = np.concatenate(
        [features, centers1, features_ood, np.zeros((PAD, D), np.float32)], axis=0
    )
    g8h = g.astype(FP8NP)
    g8l = (g - g8h.astype(np.float32)).astype(FP8NP)

    row_perm = np.argsort(targets, kind="stable")
    t_sorted = targets[row_perm]

    # per-core column permutations
    perms = []
    win_need = 1
    all_batch = np.arange(B)
    for c in range(NCORES):
        own = row_perm[RPC * c : RPC * (c + 1)]            # sorted by class
        tset = np.zeros(C + 1, bool)
        tset[t_sorted[RPC * c : RPC * (c + 1)]] = True
        in_own = np.zeros(B, bool)
        in_own[own] = True
        match_b = all_batch[tset[targets] & ~in_own]       # other cores' rows, own classes
        match_c = B + np.flatnonzero(tset[:C])             # centers of own classes
        matched = np.concatenate([match_b, match_c])
        rest_mask = np.ones(B + C, bool)
        rest_mask[own] = False
        rest_mask[matched] = False
        rest = np.flatnonzero(rest_mask)
        perm = np.concatenate(
            [own, matched, rest,
             np.arange(B + C, N),                          # ood
             np.arange(N, NPAD)]                           # pad
        )
        assert perm.shape == (NPAD,)
        perms.append(perm)
        win_need = max(win_need, RPC + len(matched))

    wch = max(2, -(-win_need // 512))
    win = 512 * wch
    cch = NCH - wch

    # window bias rows (bf16 hi+lo) and the device-effective per-class value
    cls_bias = -(np.log(w_full) + 10.0) / 10.0
    cb_h = cls_bias.astype(BFNP)
    cb_l = (cls_bias - cb_h.astype(np.float64)).astype(BFNP)
    bias_eff_cls = (cb_h.astype(np.float32) + cb_l.astype(np.float32)).astype(np.float64)

    bh_all = bias1.astype(BFNP)
    bl_all = (bias1 - bh_all.astype(np.float64)).astype(BFNP)
    b8_rows = _fp8_cascade(bias1, 3)                       # bh, bm, bl fp8

    ones8_host = np.zeros((2, 2, 128), np.float32)
    ones8_host[0, 0] = 1.0
    ones8_host[0, 1] = 1.0
    ones8_host[1, 0] = 1.0
    ones2_host = np.ones((2, 128), np.float32)
    ident = np.eye(128, dtype=np.float32)

    in_maps = []
    for c in range(NCORES):
        perm = perms[c]
        own = perm[:RPC]
        f8h = g8h[own]                                     # [512, 512] fp8
        f8l = g8l[own]
        # cheap bias rows: [p, cch_idx, i, j] with (0,0)=bh (0,1)=bm (1,0)=bl
        b8c = np.zeros((2, cch, 2, 512), FP8NP)
        pc = perm[win:].reshape(cch, 512)
        b8c[0, :, 0] = b8_rows[0][pc]
        b8c[0, :, 1] = b8_rows[1][pc]
        b8c[1, :, 0] = b8_rows[2][pc]
        bw_c = np.stack([bh_all[perm[:win]], bl_all[perm[:win]]])
        ta_p = lab[perm[:win]].astype(np.float32)
        in_maps.append(
            {
                "warm": np.full((2, 1024), 0.125, FP8NP),
                "gT8": _dr_tile(g8h[perm]),
                "gT8l": _dr_tile(g8l[perm[:win]]),
                "fT8": _dr_tile_f(f8h),
                "fT8l": _dr_tile_f(f8l),
                "ones8": np.ascontiguousarray(ones8_host.reshape(2, 256).astype(FP8NP)),
                "b8": np.ascontiguousarray(b8c.reshape(2, cch * 1024)),
                "ones2": np.ascontiguousarray(ones2_host.astype(BFNP)),
                "bw": np.ascontiguousarray(bw_c.astype(BFNP)),
                "ta": np.ascontiguousarray(np.broadcast_to(ta_p, (128, win))),
                "tvec": np.ascontiguousarray(
                    t_sorted[RPC * c : RPC * (c + 1)].reshape(MT, 128).T.astype(np.float32)
                ),
                "ident": ident,
            }
        )

    host = {"t_sorted": t_sorted, "w_full": w_full, "bias_eff_cls": bias_eff_cls,
            "wch": wch}
    return in_maps, host


def _combine(results, host):
    t_sorted = host["t_sorted"]
    w_full = host["w_full"]
    cnt_batch = np.bincount(t_sorted, minlength=C).astype(np.float64)

    A = np.empty(B)
    S2 = np.empty(B)
    S3 = np.empty(B)
    diag = np.empty(B)
    for c in range(NCORES):
        o = np.asarray(results[c]["out"], np.float64)  # [128, 16]
        for m in range(MT):
            rs = slice(RPC * c + 128 * m, RPC * c + 128 * (m + 1))
            A[rs] = o[:, 4 * m]
            S2[rs] = o[:, 4 * m + 1]
            S3[rs] = o[:, 4 * m + 2]
            diag[rs] = o[:, 4 * m + 3]

    ws = w_full[t_sorted]
    K = cnt_batch[t_sorted]
    ds_ = 1.0 / (ws - 1.0) - 1.0 / ws
    b1s = host["bias_eff_cls"][t_sorted]
    e1s = np.exp(10.0 * diag)
    S = A - e1s + ds_ * ws * (S2 - e1s)
    P = 10.0 * (S3 - K * b1s - diag) - 10.0 * K
    val = P / K - np.log(S)
    return np.float32(-val.mean())


def _run(inputs, trace=False, **kw):
    in_maps, host = _prepare(**inputs)
    nc = _get_nc(host["wch"])
    res = run_bass_kernel_spmd(nc, in_maps, list(range(NCORES)), trace=trace, **kw)
    loss = _combine(res.results, host)
    return loss, res


def kernel(**inputs):
    loss, _ = _run(inputs)
    return loss


# revision 15
# speedup vs baseline: 1.6577x; 1.1392x over previous
"""Trainium2 Bass kernel for the class-balanced supervised-contrastive loss.

Math (reference semantics, shift-invariant form with constant shift 10):
  l_ij = (f_i . g_j) / T,  T = 0.1, g = [features; centers; features_ood]
  E_ij = exp(l_ij - 10)
  S_i  = sum_{j != i} E_ij / (w_j - eq_ij)        (w_j = class count, eq = label match)
  P_i  = sum_{j != i} eq_ij (l_ij - 10)
  loss = -mean_i( P_i / K_i - log S_i ),  K_i = batch count of class t_i

Device per core (rows globally sorted by class, 512 rows/core, columns permuted
so every eq-match lands in the first WIN=1024 cols):
  psum = f . g + bias1_col   (bias1 = (ln(1/w) - 10)/10, so exp(10*psum) = E/w)
  A_i  = sum_j exp(10*psum)            ACT exp accum_out
  S2_i = sum_{win} eq * E1             DVE masked reduce (window only)
  S3_i = sum_{win} eq * psum           DVE masked reduce (window only)
  diag = psum_ii                       self column, for exclusion terms

All matmuls run as fp8e4 DoubleRow (2 contraction rows per PE pass):
  - main terms: f8h . g8h over K=512 as 2 DR passes of K_eff=256
  - window adds f8l.g8h + f8h.g8l correction terms (dot err ~2.5e-4) plus an
    exact bf16 (hi,lo) K=2 bias matmul
  - cheap (non-window) cols get their bias as a 3-row fp8 DR matmul
    (bh,bm,bl residual cascade, exponent err ~1e-2 -> A err ~1e-4), which is
    also the bank-opening start=True instruction for psum zeroing
Column space per (m-tile) is processed in [128,2048] psum groups (4 banks,
double buffered), each consumed by one wide ACT exp with accum_out.
"""

import ml_dtypes
import numpy as np

import concourse.bass as bass
import concourse.mybir as mybir
import concourse.tile as tile
from concourse.bass_utils import run_bass_kernel_spmd

NCORES = 8
C, TEMP = 1000, 0.1
B, BO, D = 4096, 4096, 512
N = B + C + BO              # 9192
NPAD = 9216                 # 18 * 512
PAD = NPAD - N
NCH = NPAD // 512           # 18 column chunks
RPC = B // NCORES           # 512 rows per core
MT = RPC // 128             # 4 row tiles per core

F32 = mybir.dt.float32
BF16 = mybir.dt.bfloat16
FP8 = mybir.dt.float8e4
DR = mybir.MatmulPerfMode.DoubleRow
ALU = mybir.AluOpType
AF = mybir.ActivationFunctionType
BFNP = ml_dtypes.bfloat16
FP8NP = ml_dtypes.float8_e4m3

# This walrus build accepts only one sync-wait command per engine instruction.
# Move surplus waits onto standalone EventSemaphore instructions just before
# the affected instruction (same engine, so blocking semantics are identical).
_SPLIT_SKIP = ("InstEventSemaphore",)


def _split_multi_waits(nc):
    n = 0
    for f in nc.m.functions:
        for bb in f.blocks:
            new = []
            for ins in bb.instructions:
                si = ins.sync_info
                if (
                    si is not None
                    and si.on_wait
                    and len(si.on_wait) > 1
                    and type(ins).__name__ not in _SPLIT_SKIP
                ):
                    waits = list(si.on_wait)
                    for w in waits[:-1]:
                        es = mybir.InstEventSemaphore(
                            name=f"wsplit_{n}",
                            engine=ins.engine,
                            sync_info=mybir.SyncInfo(on_wait=[w], on_update=[]),
                        )
                        n += 1
                        new.append(es)
                    ins.sync_info = mybir.SyncInfo(
                        on_wait=[waits[-1]], on_update=list(si.on_update)
                    )
                new.append(ins)
            bb.instructions = new
    return n


def _build_nc(wch=2):
    """wch = number of 512-col window chunks holding all eq-matches."""
    cch = NCH - wch                     # cheap chunks
    win = 512 * wch
    # cheap chunks packed into psum groups of <=4 chunks
    groups = []
    ch = wch
    while ch < NCH:
        g = list(range(ch, min(ch + 4, NCH)))
        groups.append(g)
        ch += len(g)
    ngrp = len(groups) + 1              # + window group
    nc = bass.Bass()

    # DR layouts: contraction row r = 256*khat + 128*i + p  (pair i, partition p)
    gT8 = nc.declare_dram_parameter("gT8", [128, NCH * 2048], FP8, isOutput=False)
    gT8l = nc.declare_dram_parameter("gT8l", [128, wch * 2048], FP8, isOutput=False)
    fT8 = nc.declare_dram_parameter("fT8", [128, MT * 512], FP8, isOutput=False)
    fT8l = nc.declare_dram_parameter("fT8l", [128, MT * 512], FP8, isOutput=False)
    warm = nc.declare_dram_parameter("warm", [2, 1024], FP8, isOutput=False)
    ones8 = nc.declare_dram_parameter("ones8", [2, 256], FP8, isOutput=False)
    b8 = nc.declare_dram_parameter("b8", [2, cch * 1024], FP8, isOutput=False)
    ones2 = nc.declare_dram_parameter("ones2", [2, 128], BF16, isOutput=False)
    bw = nc.declare_dram_parameter("bw", [2, win], BF16, isOutput=False)
    ta = nc.declare_dram_parameter("ta", [128, win], F32, isOutput=False)
    tvec = nc.declare_dram_parameter("tvec", [128, MT], F32, isOutput=False)
    ident = nc.declare_dram_parameter("ident", [128, 128], F32, isOutput=False)
    out = nc.declare_dram_parameter("out", [128, 3 * MT], F32, isOutput=True)

    with tile.TileContext(nc) as tc:
        with (
            tc.tile_pool(name="const", bufs=1) as const,
            tc.tile_pool(name="stats", bufs=1) as stats,
            tc.tile_pool(name="gt", bufs=8) as gtp,
            tc.tile_pool(name="e1c", bufs=3) as e1cp,
            tc.tile_pool(name="e1w", bufs=2) as e1wp,
            tc.tile_pool(name="psum", bufs=2, space="PSUM") as psp,
        ):
            ft = const.tile([128, MT, 2, 2, 128], FP8)
            ftl = const.tile([128, MT, 2, 2, 128], FP8)
            warm_sb = const.tile([2, 2, 512], FP8)
            warm_o = const.tile([2, 512], F32)
            ones8_sb = const.tile([2, 2, 128], FP8)
            b8_sb = const.tile([2, cch, 2, 512], FP8)
            ones2_sb = const.tile([2, 128], BF16)
            bw_sb = const.tile([2, win], BF16)
            ta_sb = const.tile([128, win], F32)
            tvec_sb = const.tile([128, MT], F32)
            ident_sb = const.tile([128, 128], F32)
            gl = const.tile([128, wch, 2, 2, 512], FP8)

            nc.sync.dma_start(out=warm_sb[:], in_=warm[:])
            nc.sync.dma_start(out=ones8_sb[:], in_=ones8[:])
            nc.gpsimd.dma_start(out=ft[:], in_=fT8[:])
            # Exp table preload off the critical path
            nc.scalar.activation(warm_o[:], warm_sb[:, 0], AF.Exp, scale=1.0)

            outsb = stats.tile([128, 3 * MT], F32)
            # per m: ngrp A-partials | wch S2 parts
            acc = [stats.tile([128, ngrp + wch], F32, name=f"acc{m}") for m in range(MT)]

            group_tiles = {}

            def load_group(gi, g, spread=False):
                nc.sync.dma_start(
                    out=b8_sb[:, g[0] - wch : g[-1] + 1 - wch],
                    in_=b8[:, 1024 * (g[0] - wch) : 1024 * (g[-1] + 1 - wch)],
                )
                gts = []
                for ci, ch in enumerate(g):
                    gt = gtp.tile([128, 2, 2, 512], FP8, name=f"g{ch}", tag="gt")
                    eng = nc.gpsimd if (spread and ci >= 2) else nc.sync
                    eng.dma_start(
                        out=gt[:], in_=gT8[:, 2048 * ch : 2048 * (ch + 1)]
                    )
                    gts.append(gt)
                group_tiles[gi] = gts

            def cheap_unit(gi, g, m, warmups=0):
                gts = group_tiles[gi]
                ps = psp.tile([128, 2048], F32)
                # PE warmup/p-state ramp spins while the first DMAs land
                for _ in range(warmups):
                    nc.tensor.matmul(
                        ps[:, 0:512], warm_sb[:, :, :128], warm_sb[:],
                        start=True, stop=True, perf_mode=DR, skip_group_check=True,
                    )
                for ci, ch in enumerate(g):
                    cs = slice(512 * ci, 512 * (ci + 1))
                    nc.tensor.matmul(
                        ps[:, cs], ones8_sb[:], b8_sb[:, ch - wch],
                        start=True, stop=False, perf_mode=DR,
                    )
                    for k in range(2):
                        nc.tensor.matmul(
                            ps[:, cs], ft[:, m, k], gts[ci][:, k],
                            start=False, stop=(k == 1), perf_mode=DR,
                        )
                e1c = e1cp.tile([128, 512 * len(g)], BF16, tag="e1c")
                nc.scalar.activation(
                    e1c[:], ps[:, : 512 * len(g)], AF.Exp, scale=10.0,
                    accum_out=acc[m][:, 1 + gi : 2 + gi],
                )

            load_group(0, groups[0], spread=True)
            cheap_unit(0, groups[0], 0, warmups=12)
            for m in range(1, MT):
                cheap_unit(0, groups[0], m)

            # window DMAs (idle Pool queue) land while group 0 computes
            nc.gpsimd.dma_start(out=ftl[:], in_=fT8l[:])
            nc.gpsimd.dma_start(out=ones2_sb[:], in_=ones2[:])
            nc.gpsimd.dma_start(out=bw_sb[:], in_=bw[:])
            nc.gpsimd.dma_start(out=ta_sb[:], in_=ta[:])
            nc.gpsimd.dma_start(out=tvec_sb[:], in_=tvec[:])
            nc.gpsimd.dma_start(out=ident_sb[:], in_=ident[:])
            gw = []
            for ch in range(wch):
                nc.gpsimd.dma_start(
                    out=gl[:, ch], in_=gT8l[:, 2048 * ch : 2048 * (ch + 1)]
                )
                gt = gtp.tile([128, 2, 2, 512], FP8, name=f"gw{ch}", tag="gtw")
                nc.gpsimd.dma_start(out=gt[:], in_=gT8[:, 2048 * ch : 2048 * (ch + 1)])
                gw.append(gt)

            for m in range(MT):
                ps = psp.tile([128, 2048], F32)
                for ch in range(wch):
                    cs = slice(512 * ch, 512 * (ch + 1))
                    nc.tensor.matmul(
                        ps[:, cs], ones2_sb[:], bw_sb[:, cs],
                        start=True, stop=False,
                    )
                    terms = (
                        (ft, (gw[ch][:, 0], gw[ch][:, 1])),
                        (ftl, (gw[ch][:, 0], gw[ch][:, 1])),
                        (ft, (gl[:, ch, 0], gl[:, ch, 1])),
                    )
                    for ti, (lhs, rhss) in enumerate(terms):
                        for k in range(2):
                            nc.tensor.matmul(
                                ps[:, cs], lhs[:, m, k], rhss[k],
                                start=False,
                                stop=(ti == 2 and k == 1),
                                perf_mode=DR,
                            )
                e1w = e1wp.tile([128, win], F32, tag="e1w")
                nc.scalar.activation(
                    e1w[:], ps[:, :win], AF.Exp, scale=10.0,
                    accum_out=acc[m][:, 0:1],
                )
                # self term e1s: gather the diagonal of the SBUF exp tile (no
                # psum read, so the psum buffer frees as soon as the exp ends)
                sd = e1wp.tile([128, 128], F32, tag="scrd")
                nc.vector.scalar_tensor_tensor(
                    out=sd[:], in0=ident_sb[:], scalar=1.0,
                    in1=e1w[:, 128 * m : 128 * (m + 1)],
                    op0=ALU.mult, op1=ALU.mult,
                    accum_out=outsb[:, 3 * m + 2 : 3 * m + 3],
                )
                for ch in range(wch):
                    cs = slice(512 * ch, 512 * (ch + 1))
                    sc = e1wp.tile([128, 512], F32, tag="scr2")
                    nc.vector.scalar_tensor_tensor(
                        out=sc[:], in0=ta_sb[:, cs], scalar=tvec_sb[:, m : m + 1],
                        in1=e1w[:, cs], op0=ALU.is_equal, op1=ALU.mult,
                        accum_out=acc[m][:, ngrp + ch : ngrp + ch + 1],
                    )
                # interleave one cheap unit after each window group so ACT
                # always has a 2048-wide exp queued while DVE drains the window
                if m < MT:
                    if m == 0:
                        load_group(1, groups[1])
                    cheap_unit(1, groups[1], m)

            # -- remaining cheap groups -------------------------------------
            for gi, g in enumerate(groups[2:], start=2):
                load_group(gi, g)
                for m in range(MT):
                    cheap_unit(gi, g, m)

            for m in range(MT):
                nc.vector.tensor_reduce(
                    outsb[:, 3 * m : 3 * m + 1], acc[m][:, 0:ngrp],
                    mybir.AxisListType.X, ALU.add,
                )
                nc.vector.tensor_reduce(
                    outsb[:, 3 * m + 1 : 3 * m + 2], acc[m][:, ngrp : ngrp + wch],
                    mybir.AxisListType.X, ALU.add,
                )
            nc.sync.dma_start(out=out[:], in_=outsb[:])
    _split_multi_waits(nc)
    return nc


_nc_by_cfg = {}


def _get_nc(wch):
    if wch not in _nc_by_cfg:
        _nc_by_cfg[wch] = _build_nc(wch)
    return _nc_by_cfg[wch]


def _fp8_cascade(x, n):
    """Split x into n fp8 rows summing (in f32) to ~x."""
    rows = []
    rem = np.asarray(x, np.float64).copy()
    for _ in range(n):
        h = rem.astype(FP8NP)
        rows.append(h)
        rem = rem - h.astype(np.float64)
    return rows


def _dr_tile(x):
    """[ncols, 512] fp8 -> [128, ncols/512 * 2048] in the DR chunk layout:
    [p, ch*2048 + (khat*2 + i)*512 + j] = x[512*ch + j, 256*khat + 128*i + p]."""
    nch = x.shape[0] // 512
    xt = np.ascontiguousarray(x.T)                  # [512, ncols]
    return np.ascontiguousarray(
        xt.reshape(2, 2, 128, nch, 512).transpose(2, 3, 0, 1, 4).reshape(128, -1)
    )


def _dr_tile_f(x):
    """[512 rows, 512 dims] fp8 -> [128, MT*512] stationary layout:
    [p, ((m*2 + khat)*2 + i)*128 + q] = x[128*m + q, 256*khat + 128*i + p]."""
    xt = np.ascontiguousarray(x.T)                  # [512 dims, 512 rows]
    return np.ascontiguousarray(
        xt.reshape(2, 2, 128, MT, 128).transpose(2, 3, 0, 1, 4).reshape(128, -1)
    )


def _prepare(centers1, features, targets, features_ood, pseudo_target_ood):
    """Host-side prep: sort rows by class, shard contiguously, and per core
    permute the g columns to [own 512 | matched | rest | ood | pad] so all
    eq-matches (and the diagonal, at window column 128m+p) land in the first
    WIN columns."""
    centers1 = np.asarray(centers1, np.float32)
    features = np.asarray(features, np.float32)
    features_ood = np.asarray(features_ood, np.float32)
    targets = np.asarray(targets).astype(np.int64)
    pseudo = np.asarray(pseudo_target_ood).astype(np.int64)

    tac = np.concatenate([targets, np.arange(C), pseudo])
    w_full = np.bincount(tac, minlength=C).astype(np.float64)

    # class-id label per g row (incl. centers/ood), and bias per g row
    lab = np.concatenate([targets, np.arange(C), np.full(BO, C, np.int64),
                          np.full(PAD, -1, np.int64)])
    bias1 = np.full(NPAD, -20.0, np.float64)
    bias1[:N] = -(np.log(w_full[tac]) + 10.0) / 10.0

    g = np.concatenate(
        [features, centers1, features_ood, np.zeros((PAD, D), np.float32)], axis=0
    )
    g8h = g.astype(FP8NP)
    g8l = (g - g8h.astype(np.float32)).astype(FP8NP)

    row_perm = np.argsort(targets, kind="stable")
    t_sorted = targets[row_perm]

    # per-core column permutations
    perms = []
    win_need = 1
    all_batch = np.arange(B)
    for c in range(NCORES):
        own = row_perm[RPC * c : RPC * (c + 1)]            # sorted by class
        tset = np.zeros(C + 1, bool)
        tset[t_sorted[RPC * c : RPC * (c + 1)]] = True
        in_own = np.zeros(B, bool)
        in_own[own] = True
        match_b = all_batch[tset[targets] & ~in_own]       # other cores' rows, own classes
        match_c = B + np.flatnonzero(tset[:C])             # centers of own classes
        matched = np.concatenate([match_b, match_c])
        rest_mask = np.ones(B + C, bool)
        rest_mask[own] = False
        rest_mask[matched] = False
        rest = np.flatnonzero(rest_mask)
        perm = np.concatenate(
            [own, matched, rest,
             np.arange(B + C, N),                          # ood
             np.arange(N, NPAD)]                           # pad
        )
        assert perm.shape == (NPAD,)
        perms.append(perm)
        win_need = max(win_need, RPC + len(matched))

    wch = max(2, -(-win_need // 512))
    win = 512 * wch
    cch = NCH - wch

    # window bias rows (bf16 hi+lo)
    bh_all = bias1.astype(BFNP)
    bl_all = (bias1 - bh_all.astype(np.float64)).astype(BFNP)
    b8_rows = _fp8_cascade(bias1, 3)                       # bh, bm, bl fp8

    ones8_host = np.zeros((2, 2, 128), np.float32)
    ones8_host[0, 0] = 1.0
    ones8_host[0, 1] = 1.0
    ones8_host[1, 0] = 1.0
    ones2_host = np.ones((2, 128), np.float32)
    ident = np.eye(128, dtype=np.float32)

    in_maps = []
    for c in range(NCORES):
        perm = perms[c]
        own = perm[:RPC]
        f8h = g8h[own]                                     # [512, 512] fp8
        f8l = g8l[own]
        # cheap bias rows: [p, cch_idx, i, j] with (0,0)=bh (0,1)=bm (1,0)=bl
        b8c = np.zeros((2, cch, 2, 512), FP8NP)
        pc = perm[win:].reshape(cch, 512)
        b8c[0, :, 0] = b8_rows[0][pc]
        b8c[0, :, 1] = b8_rows[1][pc]
        b8c[1, :, 0] = b8_rows[2][pc]
        bw_c = np.stack([bh_all[perm[:win]], bl_all[perm[:win]]])
        ta_p = lab[perm[:win]].astype(np.float32)
        in_maps.append(
            {
                "warm": np.full((2, 1024), 0.125, FP8NP),
                "gT8": _dr_tile(g8h[perm]),
                "gT8l": _dr_tile(g8l[perm[:win]]),
                "fT8": _dr_tile_f(f8h),
                "fT8l": _dr_tile_f(f8l),
                "ones8": np.ascontiguousarray(ones8_host.reshape(2, 256).astype(FP8NP)),
                "b8": np.ascontiguousarray(b8c.reshape(2, cch * 1024)),
                "ones2": np.ascontiguousarray(ones2_host.astype(BFNP)),
                "bw": np.ascontiguousarray(bw_c.astype(BFNP)),
                "ta": np.ascontiguousarray(np.broadcast_to(ta_p, (128, win))),
                "tvec": np.ascontiguousarray(
                    t_sorted[RPC * c : RPC * (c + 1)].reshape(MT, 128).T.astype(np.float32)
                ),
                "ident": ident,
            }
        )

    # host-side numerator: P_i = 10*(f_i . G_{t_i} - f_i . f_i) - 10*K_i
    # with G_c = sum of batch features of class c + center_c (exact, f64)
    f64 = features.astype(np.float64)
    G = centers1.astype(np.float64).copy()
    np.add.at(G, targets, f64)
    dots = np.einsum("ij,ij->i", f64, G[targets])
    self_dot = np.einsum("ij,ij->i", f64, f64)
    P_half = dots - self_dot                       # sum over matched != self of r

    host = {"t_sorted": t_sorted, "w_full": w_full, "wch": wch,
            "P_half": P_half[row_perm]}
    return in_maps, host


def _combine(results, host):
    t_sorted = host["t_sorted"]
    w_full = host["w_full"]
    cnt_batch = np.bincount(t_sorted, minlength=C).astype(np.float64)

    A = np.empty(B)
    S2 = np.empty(B)
    e1s = np.empty(B)
    for c in range(NCORES):
        o = np.asarray(results[c]["out"], np.float64)  # [128, 12]
        for m in range(MT):
            rs = slice(RPC * c + 128 * m, RPC * c + 128 * (m + 1))
            A[rs] = o[:, 3 * m]
            S2[rs] = o[:, 3 * m + 1]
            e1s[rs] = o[:, 3 * m + 2]

    ws = w_full[t_sorted]
    K = cnt_batch[t_sorted]
    ds_ = 1.0 / (ws - 1.0) - 1.0 / ws
    S = A - e1s + ds_ * ws * (S2 - e1s)
    P = 10.0 * host["P_half"] - 10.0 * K
    val = P / K - np.log(S)
    return np.float32(-val.mean())


def _run(inputs, trace=False, **kw):
    in_maps, host = _prepare(**inputs)
    nc = _get_nc(host["wch"])
    res = run_bass_kernel_spmd(nc, in_maps, list(range(NCORES)), trace=trace, **kw)
    loss = _combine(res.results, host)
    return loss, res


def kernel(**inputs):
    loss, _ = _run(inputs)
    return loss


# revision 16
# speedup vs baseline: 1.6782x; 1.0124x over previous
"""Trainium2 Bass kernel for the class-balanced supervised-contrastive loss.

Math (reference semantics, shift-invariant form with constant shift 10):
  l_ij = (f_i . g_j) / T,  T = 0.1, g = [features; centers; features_ood]
  E_ij = exp(l_ij - 10)
  S_i  = sum_{j != i} E_ij / (w_j - eq_ij)        (w_j = class count, eq = label match)
  P_i  = sum_{j != i} eq_ij (l_ij - 10)
  loss = -mean_i( P_i / K_i - log S_i ),  K_i = batch count of class t_i

Device per core (rows globally sorted by class, 512 rows/core, columns permuted
so every eq-match lands in the first WIN=1024 cols):
  psum = f . g + bias1_col   (bias1 = (ln(1/w) - 10)/10, so exp(10*psum) = E/w)
  A_i  = sum_j exp(10*psum)            ACT exp accum_out
  S2_i = sum_{win} eq * E1             DVE masked reduce (window only)
  S3_i = sum_{win} eq * psum           DVE masked reduce (window only)
  diag = psum_ii                       self column, for exclusion terms

All matmuls run as fp8e4 DoubleRow (2 contraction rows per PE pass):
  - main terms: f8h . g8h over K=512 as 2 DR passes of K_eff=256
  - window adds f8l.g8h + f8h.g8l correction terms (dot err ~2.5e-4) plus an
    exact bf16 (hi,lo) K=2 bias matmul
  - cheap (non-window) cols get their bias as a 3-row fp8 DR matmul
    (bh,bm,bl residual cascade, exponent err ~1e-2 -> A err ~1e-4), which is
    also the bank-opening start=True instruction for psum zeroing
Column space per (m-tile) is processed in [128,2048] psum groups (4 banks,
double buffered), each consumed by one wide ACT exp with accum_out.
"""

import ml_dtypes
import numpy as np

import concourse.bass as bass
import concourse.mybir as mybir
import concourse.tile as tile
from concourse.bass_utils import run_bass_kernel_spmd

NCORES = 8
C, TEMP = 1000, 0.1
B, BO, D = 4096, 4096, 512
N = B + C + BO              # 9192
NPAD = 9216                 # 18 * 512
PAD = NPAD - N
NCH = NPAD // 512           # 18 column chunks
RPC = B // NCORES           # 512 rows per core
MT = RPC // 128             # 4 row tiles per core

F32 = mybir.dt.float32
BF16 = mybir.dt.bfloat16
FP8 = mybir.dt.float8e4
DR = mybir.MatmulPerfMode.DoubleRow
ALU = mybir.AluOpType
AF = mybir.ActivationFunctionType
BFNP = ml_dtypes.bfloat16
FP8NP = ml_dtypes.float8_e4m3

# This walrus build accepts only one sync-wait command per engine instruction.
# Move surplus waits onto standalone EventSemaphore instructions just before
# the affected instruction (same engine, so blocking semantics are identical).
_SPLIT_SKIP = ("InstEventSemaphore",)


def _split_multi_waits(nc):
    n = 0
    for f in nc.m.functions:
        for bb in f.blocks:
            new = []
            for ins in bb.instructions:
                si = ins.sync_info
                if (
                    si is not None
                    and si.on_wait
                    and len(si.on_wait) > 1
                    and type(ins).__name__ not in _SPLIT_SKIP
                ):
                    waits = list(si.on_wait)
                    for w in waits[:-1]:
                        es = mybir.InstEventSemaphore(
                            name=f"wsplit_{n}",
                            engine=ins.engine,
                            sync_info=mybir.SyncInfo(on_wait=[w], on_update=[]),
                        )
                        n += 1
                        new.append(es)
                    ins.sync_info = mybir.SyncInfo(
                        on_wait=[waits[-1]], on_update=list(si.on_update)
                    )
                new.append(ins)
            bb.instructions = new
    return n


def _build_nc(wch=2):
    """wch = number of 512-col window chunks holding all eq-matches."""
    cch = NCH - wch                     # cheap chunks
    win = 512 * wch
    # cheap chunks packed into psum groups of <=4 chunks
    groups = []
    ch = wch
    while ch < NCH:
        g = list(range(ch, min(ch + 4, NCH)))
        groups.append(g)
        ch += len(g)
    ngrp = len(groups) + 1              # + window group
    nc = bass.Bass()

    # DR layouts: contraction row r = 256*khat + 128*i + p  (pair i, partition p)
    gT8 = nc.declare_dram_parameter("gT8", [128, NCH * 2048], FP8, isOutput=False)
    gT8l = nc.declare_dram_parameter("gT8l", [128, wch * 2048], FP8, isOutput=False)
    fT8 = nc.declare_dram_parameter("fT8", [128, MT * 512], FP8, isOutput=False)
    fT8l = nc.declare_dram_parameter("fT8l", [128, MT * 512], FP8, isOutput=False)
    warm = nc.declare_dram_parameter("warm", [2, 1024], FP8, isOutput=False)
    ones8 = nc.declare_dram_parameter("ones8", [2, 256], FP8, isOutput=False)
    b8 = nc.declare_dram_parameter("b8", [2, cch * 1024], FP8, isOutput=False)
    ones2 = nc.declare_dram_parameter("ones2", [2, 128], BF16, isOutput=False)
    bw = nc.declare_dram_parameter("bw", [2, win], BF16, isOutput=False)
    ta = nc.declare_dram_parameter("ta", [128, win], F32, isOutput=False)
    tvec = nc.declare_dram_parameter("tvec", [128, MT], F32, isOutput=False)
    ident = nc.declare_dram_parameter("ident", [128, 128], F32, isOutput=False)
    out = nc.declare_dram_parameter("out", [128, (len(groups) + 1 + wch + 1) * MT], F32, isOutput=True)

    with tile.TileContext(nc) as tc:
        with (
            tc.tile_pool(name="const", bufs=1) as const,
            tc.tile_pool(name="stats", bufs=1) as stats,
            tc.tile_pool(name="gt", bufs=8) as gtp,
            tc.tile_pool(name="e1c", bufs=3) as e1cp,
            tc.tile_pool(name="e1w", bufs=2) as e1wp,
            tc.tile_pool(name="psum", bufs=2, space="PSUM") as psp,
        ):
            ft = const.tile([128, MT, 2, 2, 128], FP8)
            ftl = const.tile([128, MT, 2, 2, 128], FP8)
            warm_sb = const.tile([2, 2, 512], FP8)
            warm_o = const.tile([2, 512], F32)
            ones8_sb = const.tile([2, 2, 128], FP8)
            b8_sb = const.tile([2, cch, 2, 512], FP8)
            ones2_sb = const.tile([2, 128], BF16)
            bw_sb = const.tile([2, win], BF16)
            ta_sb = const.tile([128, win], F32)
            tvec_sb = const.tile([128, MT], F32)
            ident_sb = const.tile([128, 128], F32)
            gl = const.tile([128, wch, 2, 2, 512], FP8)

            nc.sync.dma_start(out=warm_sb[:], in_=warm[:])
            nc.sync.dma_start(out=ones8_sb[:], in_=ones8[:])
            nc.gpsimd.dma_start(out=ft[:], in_=fT8[:])
            # Exp table preload off the critical path
            nc.scalar.activation(warm_o[:], warm_sb[:, 0], AF.Exp, scale=1.0)

            # per m: ngrp A-partials | wch S2 parts | e1s diag
            acc = [stats.tile([128, ngrp + wch + 1], F32, name=f"acc{m}") for m in range(MT)]

            group_tiles = {}

            def load_group(gi, g, spread=False):
                nc.sync.dma_start(
                    out=b8_sb[:, g[0] - wch : g[-1] + 1 - wch],
                    in_=b8[:, 1024 * (g[0] - wch) : 1024 * (g[-1] + 1 - wch)],
                )
                gts = []
                for ci, ch in enumerate(g):
                    gt = gtp.tile([128, 2, 2, 512], FP8, name=f"g{ch}", tag="gt")
                    eng = nc.sync
                    if spread:
                        eng = (nc.sync, nc.scalar, nc.gpsimd, nc.sync)[ci % 4]
                    eng.dma_start(
                        out=gt[:], in_=gT8[:, 2048 * ch : 2048 * (ch + 1)]
                    )
                    gts.append(gt)
                group_tiles[gi] = gts

            def cheap_unit(gi, g, m, warmups=0):
                gts = group_tiles[gi]
                ps = psp.tile([128, 2048], F32)
                # PE warmup/p-state ramp spins while the first DMAs land
                for _ in range(warmups):
                    nc.tensor.matmul(
                        ps[:, 0:512], warm_sb[:, :, :128], warm_sb[:],
                        start=True, stop=True, perf_mode=DR, skip_group_check=True,
                    )
                for ci, ch in enumerate(g):
                    cs = slice(512 * ci, 512 * (ci + 1))
                    nc.tensor.matmul(
                        ps[:, cs], ones8_sb[:], b8_sb[:, ch - wch],
                        start=True, stop=False, perf_mode=DR,
                    )
                    for k in range(2):
                        nc.tensor.matmul(
                            ps[:, cs], ft[:, m, k], gts[ci][:, k],
                            start=False, stop=(k == 1), perf_mode=DR,
                        )
                e1c = e1cp.tile([128, 512 * len(g)], BF16, tag="e1c")
                nc.scalar.activation(
                    e1c[:], ps[:, : 512 * len(g)], AF.Exp, scale=10.0,
                    accum_out=acc[m][:, 1 + gi : 2 + gi],
                )

            load_group(0, groups[0], spread=True)
            cheap_unit(0, groups[0], 0, warmups=9)
            for m in range(1, MT):
                cheap_unit(0, groups[0], m)

            # window DMAs (idle Pool queue) land while group 0 computes
            nc.gpsimd.dma_start(out=ftl[:], in_=fT8l[:])
            nc.gpsimd.dma_start(out=ones2_sb[:], in_=ones2[:])
            nc.gpsimd.dma_start(out=bw_sb[:], in_=bw[:])
            nc.gpsimd.dma_start(out=ta_sb[:], in_=ta[:])
            nc.gpsimd.dma_start(out=tvec_sb[:], in_=tvec[:])
            nc.gpsimd.dma_start(out=ident_sb[:], in_=ident[:])
            gw = []
            for ch in range(wch):
                nc.gpsimd.dma_start(
                    out=gl[:, ch], in_=gT8l[:, 2048 * ch : 2048 * (ch + 1)]
                )
                gt = gtp.tile([128, 2, 2, 512], FP8, name=f"gw{ch}", tag="gtw")
                nc.gpsimd.dma_start(out=gt[:], in_=gT8[:, 2048 * ch : 2048 * (ch + 1)])
                gw.append(gt)

            for m in range(MT):
                ps = psp.tile([128, 2048], F32)
                for ch in range(wch):
                    cs = slice(512 * ch, 512 * (ch + 1))
                    nc.tensor.matmul(
                        ps[:, cs], ones2_sb[:], bw_sb[:, cs],
                        start=True, stop=False,
                    )
                    terms = (
                        (ft, (gw[ch][:, 0], gw[ch][:, 1])),
                        (ftl, (gw[ch][:, 0], gw[ch][:, 1])),
                        (ft, (gl[:, ch, 0], gl[:, ch, 1])),
                    )
                    for ti, (lhs, rhss) in enumerate(terms):
                        for k in range(2):
                            nc.tensor.matmul(
                                ps[:, cs], lhs[:, m, k], rhss[k],
                                start=False,
                                stop=(ti == 2 and k == 1),
                                perf_mode=DR,
                            )
                e1w = e1wp.tile([128, win], F32, tag="e1w")
                nc.scalar.activation(
                    e1w[:], ps[:, :win], AF.Exp, scale=10.0,
                    accum_out=acc[m][:, 0:1],
                )
                # self term e1s: gather the diagonal of the SBUF exp tile (no
                # psum read, so the psum buffer frees as soon as the exp ends)
                sd = e1wp.tile([128, 128], F32, tag="scrd")
                nc.vector.scalar_tensor_tensor(
                    out=sd[:], in0=ident_sb[:], scalar=1.0,
                    in1=e1w[:, 128 * m : 128 * (m + 1)],
                    op0=ALU.mult, op1=ALU.mult,
                    accum_out=acc[m][:, ngrp + wch : ngrp + wch + 1],
                )
                for ch in range(wch):
                    cs = slice(512 * ch, 512 * (ch + 1))
                    sc = e1wp.tile([128, 512], F32, tag="scr2")
                    nc.vector.scalar_tensor_tensor(
                        out=sc[:], in0=ta_sb[:, cs], scalar=tvec_sb[:, m : m + 1],
                        in1=e1w[:, cs], op0=ALU.is_equal, op1=ALU.mult,
                        accum_out=acc[m][:, ngrp + ch : ngrp + ch + 1],
                    )
                # interleave one cheap unit after each window group so ACT
                # always has a 2048-wide exp queued while DVE drains the window
                if m < MT:
                    if m == 0:
                        load_group(1, groups[1])
                    cheap_unit(1, groups[1], m)

            # -- remaining cheap groups -------------------------------------
            for gi, g in enumerate(groups[2:], start=2):
                load_group(gi, g)
                for m in range(MT):
                    cheap_unit(gi, g, m)

            na = ngrp + wch + 1
            for m in range(MT):
                nc.sync.dma_start(out=out[:, na * m : na * (m + 1)], in_=acc[m][:])
    _split_multi_waits(nc)
    return nc


_nc_by_cfg = {}


def _get_nc(wch):
    if wch not in _nc_by_cfg:
        _nc_by_cfg[wch] = _build_nc(wch)
    return _nc_by_cfg[wch]


def _fp8_cascade(x, n):
    """Split x into n fp8 rows summing (in f32) to ~x."""
    rows = []
    rem = np.asarray(x, np.float64).copy()
    for _ in range(n):
        h = rem.astype(FP8NP)
        rows.append(h)
        rem = rem - h.astype(np.float64)
    return rows


def _dr_tile(x):
    """[ncols, 512] fp8 -> [128, ncols/512 * 2048] in the DR chunk layout:
    [p, ch*2048 + (khat*2 + i)*512 + j] = x[512*ch + j, 256*khat + 128*i + p]."""
    nch = x.shape[0] // 512
    xt = np.ascontiguousarray(x.T)                  # [512, ncols]
    return np.ascontiguousarray(
        xt.reshape(2, 2, 128, nch, 512).transpose(2, 3, 0, 1, 4).reshape(128, -1)
    )


def _dr_tile_f(x):
    """[512 rows, 512 dims] fp8 -> [128, MT*512] stationary layout:
    [p, ((m*2 + khat)*2 + i)*128 + q] = x[128*m + q, 256*khat + 128*i + p]."""
    xt = np.ascontiguousarray(x.T)                  # [512 dims, 512 rows]
    return np.ascontiguousarray(
        xt.reshape(2, 2, 128, MT, 128).transpose(2, 3, 0, 1, 4).reshape(128, -1)
    )


def _prepare(centers1, features, targets, features_ood, pseudo_target_ood):
    """Host-side prep: sort rows by class, shard contiguously, and per core
    permute the g columns to [own 512 | matched | rest | ood | pad] so all
    eq-matches (and the diagonal, at window column 128m+p) land in the first
    WIN columns."""
    centers1 = np.asarray(centers1, np.float32)
    features = np.asarray(features, np.float32)
    features_ood = np.asarray(features_ood, np.float32)
    targets = np.asarray(targets).astype(np.int64)
    pseudo = np.asarray(pseudo_target_ood).astype(np.int64)

    tac = np.concatenate([targets, np.arange(C), pseudo])
    w_full = np.bincount(tac, minlength=C).astype(np.float64)

    # class-id label per g row (incl. centers/ood), and bias per g row
    lab = np.concatenate([targets, np.arange(C), np.full(BO, C, np.int64),
                          np.full(PAD, -1, np.int64)])
    bias1 = np.full(NPAD, -20.0, np.float64)
    bias1[:N] = -(np.log(w_full[tac]) + 10.0) / 10.0

    g = np.concatenate(
        [features, centers1, features_ood, np.zeros((PAD, D), np.float32)], axis=0
    )
    g8h = g.astype(FP8NP)
    g8l = (g - g8h.astype(np.float32)).astype(FP8NP)

    row_perm = np.argsort(targets, kind="stable")
    t_sorted = targets[row_perm]

    # per-core column permutations
    perms = []
    win_need = 1
    all_batch = np.arange(B)
    for c in range(NCORES):
        own = row_perm[RPC * c : RPC * (c + 1)]            # sorted by class
        tset = np.zeros(C + 1, bool)
        tset[t_sorted[RPC * c : RPC * (c + 1)]] = True
        in_own = np.zeros(B, bool)
        in_own[own] = True
        match_b = all_batch[tset[targets] & ~in_own]       # other cores' rows, own classes
        match_c = B + np.flatnonzero(tset[:C])             # centers of own classes
        matched = np.concatenate([match_b, match_c])
        rest_mask = np.ones(B + C, bool)
        rest_mask[own] = False
        rest_mask[matched] = False
        rest = np.flatnonzero(rest_mask)
        perm = np.concatenate(
            [own, matched, rest,
             np.arange(B + C, N),                          # ood
             np.arange(N, NPAD)]                           # pad
        )
        assert perm.shape == (NPAD,)
        perms.append(perm)
        win_need = max(win_need, RPC + len(matched))

    wch = max(2, -(-win_need // 512))
    win = 512 * wch
    cch = NCH - wch

    # window bias rows (bf16 hi+lo)
    bh_all = bias1.astype(BFNP)
    bl_all = (bias1 - bh_all.astype(np.float64)).astype(BFNP)
    b8_rows = _fp8_cascade(bias1, 3)                       # bh, bm, bl fp8

    ones8_host = np.zeros((2, 2, 128), np.float32)
    ones8_host[0, 0] = 1.0
    ones8_host[0, 1] = 1.0
    ones8_host[1, 0] = 1.0
    ones2_host = np.ones((2, 128), np.float32)
    ident = np.eye(128, dtype=np.float32)

    in_maps = []
    for c in range(NCORES):
        perm = perms[c]
        own = perm[:RPC]
        f8h = g8h[own]                                     # [512, 512] fp8
        f8l = g8l[own]
        # cheap bias rows: [p, cch_idx, i, j] with (0,0)=bh (0,1)=bm (1,0)=bl
        b8c = np.zeros((2, cch, 2, 512), FP8NP)
        pc = perm[win:].reshape(cch, 512)
        b8c[0, :, 0] = b8_rows[0][pc]
        b8c[0, :, 1] = b8_rows[1][pc]
        b8c[1, :, 0] = b8_rows[2][pc]
        bw_c = np.stack([bh_all[perm[:win]], bl_all[perm[:win]]])
        ta_p = lab[perm[:win]].astype(np.float32)
        in_maps.append(
            {
                "warm": np.full((2, 1024), 0.125, FP8NP),
                "gT8": _dr_tile(g8h[perm]),
                "gT8l": _dr_tile(g8l[perm[:win]]),
                "fT8": _dr_tile_f(f8h),
                "fT8l": _dr_tile_f(f8l),
                "ones8": np.ascontiguousarray(ones8_host.reshape(2, 256).astype(FP8NP)),
                "b8": np.ascontiguousarray(b8c.reshape(2, cch * 1024)),
                "ones2": np.ascontiguousarray(ones2_host.astype(BFNP)),
                "bw": np.ascontiguousarray(bw_c.astype(BFNP)),
                "ta": np.ascontiguousarray(np.broadcast_to(ta_p, (128, win))),
                "tvec": np.ascontiguousarray(
                    t_sorted[RPC * c : RPC * (c + 1)].reshape(MT, 128).T.astype(np.float32)
                ),
                "ident": ident,
            }
        )

    # host-side numerator: P_i = 10*(f_i . G_{t_i} - f_i . f_i) - 10*K_i
    # with G_c = sum of batch features of class c + center_c (exact, f64)
    f64 = features.astype(np.float64)
    G = centers1.astype(np.float64).copy()
    np.add.at(G, targets, f64)
    dots = np.einsum("ij,ij->i", f64, G[targets])
    self_dot = np.einsum("ij,ij->i", f64, f64)
    P_half = dots - self_dot                       # sum over matched != self of r

    ncheap_groups = -(-(NCH - wch) // 4)
    host = {"t_sorted": t_sorted, "w_full": w_full, "wch": wch,
            "ngrp": ncheap_groups + 1, "P_half": P_half[row_perm]}
    return in_maps, host


def _combine(results, host):
    t_sorted = host["t_sorted"]
    w_full = host["w_full"]
    cnt_batch = np.bincount(t_sorted, minlength=C).astype(np.float64)

    ngrp = host["ngrp"]
    wch = host["wch"]
    na = ngrp + wch + 1
    A = np.empty(B)
    S2 = np.empty(B)
    e1s = np.empty(B)
    for c in range(NCORES):
        o = np.asarray(results[c]["out"], np.float64)
        for m in range(MT):
            rs = slice(RPC * c + 128 * m, RPC * c + 128 * (m + 1))
            a = o[:, na * m : na * (m + 1)]
            A[rs] = a[:, 0:ngrp].sum(axis=1)
            S2[rs] = a[:, ngrp : ngrp + wch].sum(axis=1)
            e1s[rs] = a[:, ngrp + wch]

    ws = w_full[t_sorted]
    K = cnt_batch[t_sorted]
    ds_ = 1.0 / (ws - 1.0) - 1.0 / ws
    S = A - e1s + ds_ * ws * (S2 - e1s)
    P = 10.0 * host["P_half"] - 10.0 * K
    val = P / K - np.log(S)
    return np.float32(-val.mean())


def _run(inputs, trace=False, **kw):
    in_maps, host = _prepare(**inputs)
    nc = _get_nc(host["wch"])
    res = run_bass_kernel_spmd(nc, in_maps, list(range(NCORES)), trace=trace, **kw)
    loss = _combine(res.results, host)
    return loss, res


def kernel(**inputs):
    loss, _ = _run(inputs)
    return loss


# revision 19
# speedup vs baseline: 1.7546x; 1.0455x over previous
"""Trainium2 Bass kernel for the class-balanced supervised-contrastive loss.

Math (reference semantics, shift-invariant form with constant shift 10):
  l_ij = (f_i . g_j) / T,  T = 0.1, g = [features; centers; features_ood]
  E_ij = exp(l_ij - 10)
  S_i  = sum_{j != i} E_ij / (w_j - eq_ij)        (w_j = class count, eq = label match)
  P_i  = sum_{j != i} eq_ij (l_ij - 10)
  loss = -mean_i( P_i / K_i - log S_i ),  K_i = batch count of class t_i

Device per core (rows globally sorted by class, 512 rows/core, columns permuted
so every eq-match lands in the first WIN=1024 cols):
  psum = f . g + bias1_col   (bias1 = (ln(1/w) - 10)/10, so exp(10*psum) = E/w)
  A_i  = sum_j exp(10*psum)            ACT exp accum_out
  S2_i = sum_{win} eq * E1             DVE masked reduce (window only)
  S3_i = sum_{win} eq * psum           DVE masked reduce (window only)
  diag = psum_ii                       self column, for exclusion terms

All matmuls run as fp8e4 DoubleRow (2 contraction rows per PE pass):
  - main terms: f8h . g8h over K=512 as 2 DR passes of K_eff=256
  - window adds f8l.g8h + f8h.g8l correction terms (dot err ~2.5e-4) plus an
    exact bf16 (hi,lo) K=2 bias matmul
  - cheap (non-window) cols get their bias as a 3-row fp8 DR matmul
    (bh,bm,bl residual cascade, exponent err ~1e-2 -> A err ~1e-4), which is
    also the bank-opening start=True instruction for psum zeroing
Column space per (m-tile) is processed in [128,2048] psum groups (4 banks,
double buffered), each consumed by one wide ACT exp with accum_out.
"""

import ml_dtypes
import numpy as np

import concourse.bass as bass
import concourse.mybir as mybir
import concourse.tile as tile
from concourse.bass_utils import run_bass_kernel_spmd

NCORES = 8
C, TEMP = 1000, 0.1
B, BO, D = 4096, 4096, 512
N = B + C + BO              # 9192
NPAD = 9216                 # 18 * 512
PAD = NPAD - N
NCH = NPAD // 512           # 18 column chunks
RPC = B // NCORES           # 512 rows per core
MT = RPC // 128             # 4 row tiles per core

F32 = mybir.dt.float32
BF16 = mybir.dt.bfloat16
FP8 = mybir.dt.float8e4
DR = mybir.MatmulPerfMode.DoubleRow
ALU = mybir.AluOpType
AF = mybir.ActivationFunctionType
BFNP = ml_dtypes.bfloat16
FP8NP = ml_dtypes.float8_e4m3

# This walrus build accepts only one sync-wait command per engine instruction.
# Move surplus waits onto standalone EventSemaphore instructions just before
# the affected instruction (same engine, so blocking semantics are identical).
_SPLIT_SKIP = ("InstEventSemaphore",)


def _split_multi_waits(nc):
    n = 0
    for f in nc.m.functions:
        for bb in f.blocks:
            new = []
            for ins in bb.instructions:
                si = ins.sync_info
                if (
                    si is not None
                    and si.on_wait
                    and len(si.on_wait) > 1
                    and type(ins).__name__ not in _SPLIT_SKIP
                ):
                    waits = list(si.on_wait)
                    for w in waits[:-1]:
                        es = mybir.InstEventSemaphore(
                            name=f"wsplit_{n}",
                            engine=ins.engine,
                            sync_info=mybir.SyncInfo(on_wait=[w], on_update=[]),
                        )
                        n += 1
                        new.append(es)
                    ins.sync_info = mybir.SyncInfo(
                        on_wait=[waits[-1]], on_update=list(si.on_update)
                    )
                new.append(ins)
            bb.instructions = new
    return n


def _build_nc(wch=2):
    """wch = number of 512-col window chunks holding all eq-matches."""
    cch = NCH - wch                     # cheap chunks
    win = 512 * wch
    # cheap chunks packed into psum groups of <=4 chunks
    groups = []
    ch = wch
    while ch < NCH:
        g = list(range(ch, min(ch + 4, NCH)))
        groups.append(g)
        ch += len(g)
    ngrp = len(groups) + 1              # + window group
    nc = bass.Bass()

    # DR layouts: contraction row r = 256*khat + 128*i + p  (pair i, partition p)
    gT8 = nc.declare_dram_parameter("gT8", [128, NCH * 2048], FP8, isOutput=False)
    gT8l = nc.declare_dram_parameter("gT8l", [128, wch * 2048], FP8, isOutput=False)
    fT8 = nc.declare_dram_parameter("fT8", [128, MT * 512], FP8, isOutput=False)
    fT8l = nc.declare_dram_parameter("fT8l", [128, MT * 512], FP8, isOutput=False)
    warm = nc.declare_dram_parameter("warm", [2, 64], FP8, isOutput=False)
    ones8 = nc.declare_dram_parameter("ones8", [2, 256], FP8, isOutput=False)
    b8 = nc.declare_dram_parameter("b8", [2, cch * 1024], FP8, isOutput=False)
    ones2 = nc.declare_dram_parameter("ones2", [2, 128], BF16, isOutput=False)
    bw = nc.declare_dram_parameter("bw", [2, win], BF16, isOutput=False)
    ta = nc.declare_dram_parameter("ta", [128, win], F32, isOutput=False)
    tvec = nc.declare_dram_parameter("tvec", [128, MT], F32, isOutput=False)
    ident = nc.declare_dram_parameter("ident", [128, 128], F32, isOutput=False)
    out = nc.declare_dram_parameter("out", [128, (len(groups) + 1 + wch + 1) * MT], F32, isOutput=True)

    with tile.TileContext(nc) as tc:
        with (
            tc.tile_pool(name="const", bufs=1) as const,
            tc.tile_pool(name="stats", bufs=1) as stats,
            tc.tile_pool(name="gt", bufs=8) as gtp,
            tc.tile_pool(name="e1c", bufs=3) as e1cp,
            tc.tile_pool(name="e1w", bufs=2) as e1wp,
            tc.tile_pool(name="psum", bufs=2, space="PSUM") as psp,
        ):
            ft = const.tile([128, MT, 2, 2, 128], FP8)
            ftl = const.tile([128, MT, 2, 2, 128], FP8)
            warm_sb = const.tile([2, 2, 32], FP8)
            warm_o = const.tile([2, 32], F32)
            ones8_sb = const.tile([2, 2, 128], FP8)
            b8_sb = const.tile([2, cch, 2, 512], FP8)
            ones2_sb = const.tile([2, 128], BF16)
            bw_sb = const.tile([2, win], BF16)
            ta_sb = const.tile([128, win], F32)
            tvec_sb = const.tile([128, MT], F32)
            ident_sb = const.tile([128, 128], F32)
            gl = const.tile([128, wch, 2, 2, 512], FP8)

            nc.sync.dma_start(out=warm_sb[:], in_=warm[:])
            nc.scalar.dma_start(out=ones8_sb[:], in_=ones8[:])
            nc.gpsimd.dma_start(out=ft[:], in_=fT8[:])
            # Exp table preload off the critical path
            nc.scalar.activation(warm_o[:], warm_sb[:, 0], AF.Exp, scale=1.0)

            # per m: ngrp A-partials | wch S2 parts | e1s diag
            acc = [stats.tile([128, ngrp + wch + 1], F32, name=f"acc{m}") for m in range(MT)]

            group_tiles = {}

            def load_group(gi, g, spread=False):
                beng = nc.gpsimd if spread else nc.sync
                beng.dma_start(
                    out=b8_sb[:, g[0] - wch : g[-1] + 1 - wch],
                    in_=b8[:, 1024 * (g[0] - wch) : 1024 * (g[-1] + 1 - wch)],
                )
                gts = []
                for ci, ch in enumerate(g):
                    gt = gtp.tile([128, 2, 2, 512], FP8, name=f"g{ch}", tag="gt")
                    eng = nc.sync
                    if spread:
                        eng = (nc.sync, nc.sync, nc.sync, nc.gpsimd)[ci % 4]
                    eng.dma_start(
                        out=gt[:], in_=gT8[:, 2048 * ch : 2048 * (ch + 1)]
                    )
                    gts.append(gt)
                group_tiles[gi] = gts

            def cheap_unit(gi, g, m, warmups=0):
                gts = group_tiles[gi]
                ps = psp.tile([128, 2048], F32)
                # PE warmup/p-state ramp spins while the first DMAs land
                for _ in range(warmups):
                    nc.tensor.matmul(
                        ps[:, 0:128], ones8_sb[:], ones8_sb[:],
                        start=True, stop=True, perf_mode=DR, skip_group_check=True,
                    )
                for ci, ch in enumerate(g):
                    cs = slice(512 * ci, 512 * (ci + 1))
                    nc.tensor.matmul(
                        ps[:, cs], ones8_sb[:], b8_sb[:, ch - wch],
                        start=True, stop=False, perf_mode=DR,
                    )
                    for k in range(2):
                        nc.tensor.matmul(
                            ps[:, cs], ft[:, m, k], gts[ci][:, k],
                            start=False, stop=(k == 1), perf_mode=DR,
                        )
                e1c = e1cp.tile([128, 512 * len(g)], BF16, tag="e1c")
                nc.scalar.activation(
                    e1c[:], ps[:, : 512 * len(g)], AF.Exp, scale=10.0,
                    accum_out=acc[m][:, 1 + gi : 2 + gi],
                )

            load_group(0, groups[0], spread=True)
            cheap_unit(0, groups[0], 0, warmups=2)
            for m in range(1, MT):
                cheap_unit(0, groups[0], m)

            # window DMAs land while group 0/1 compute
            nc.scalar.dma_start(out=ftl[:], in_=fT8l[:])
            nc.scalar.dma_start(out=ones2_sb[:], in_=ones2[:])
            nc.scalar.dma_start(out=bw_sb[:], in_=bw[:])
            nc.gpsimd.dma_start(out=ta_sb[:], in_=ta[:])
            nc.gpsimd.dma_start(out=tvec_sb[:], in_=tvec[:])
            nc.gpsimd.dma_start(out=ident_sb[:], in_=ident[:])
            gw = []
            for ch in range(wch):
                nc.gpsimd.dma_start(
                    out=gl[:, ch], in_=gT8l[:, 2048 * ch : 2048 * (ch + 1)]
                )
                gt = gtp.tile([128, 2, 2, 512], FP8, name=f"gw{ch}", tag="gtw")
                nc.gpsimd.dma_start(out=gt[:], in_=gT8[:, 2048 * ch : 2048 * (ch + 1)])
                gw.append(gt)

            load_group(1, groups[1])
            for m in range(MT):
                # interleave: cheap unit first so its psum fill leads the pair
                cheap_unit(1, groups[1], m)
                ps = psp.tile([128, 2048], F32)
                for ch in range(wch):
                    cs = slice(512 * ch, 512 * (ch + 1))
                    nc.tensor.matmul(
                        ps[:, cs], ones2_sb[:], bw_sb[:, cs],
                        start=True, stop=False,
                    )
                    terms = (
                        (ft, (gw[ch][:, 0], gw[ch][:, 1])),
                        (ftl, (gw[ch][:, 0], gw[ch][:, 1])),
                        (ft, (gl[:, ch, 0], gl[:, ch, 1])),
                    )
                    for ti, (lhs, rhss) in enumerate(terms):
                        for k in range(2):
                            nc.tensor.matmul(
                                ps[:, cs], lhs[:, m, k], rhss[k],
                                start=False,
                                stop=(ti == 2 and k == 1),
                                perf_mode=DR,
                            )
                e1w = e1wp.tile([128, win], F32, tag="e1w")
                nc.scalar.activation(
                    e1w[:], ps[:, :win], AF.Exp, scale=10.0,
                    accum_out=acc[m][:, 0:1],
                )
                # self term e1s: gather the diagonal of the SBUF exp tile (no
                # psum read, so the psum buffer frees as soon as the exp ends)
                sd = e1wp.tile([128, 128], F32, tag="scrd")
                nc.vector.scalar_tensor_tensor(
                    out=sd[:], in0=ident_sb[:], scalar=1.0,
                    in1=e1w[:, 128 * m : 128 * (m + 1)],
                    op0=ALU.mult, op1=ALU.mult,
                    accum_out=acc[m][:, ngrp + wch : ngrp + wch + 1],
                )
                for ch in range(wch):
                    cs = slice(512 * ch, 512 * (ch + 1))
                    sc = e1wp.tile([128, 512], F32, tag="scr2")
                    nc.vector.scalar_tensor_tensor(
                        out=sc[:], in0=ta_sb[:, cs], scalar=tvec_sb[:, m : m + 1],
                        in1=e1w[:, cs], op0=ALU.is_equal, op1=ALU.mult,
                        accum_out=acc[m][:, ngrp + ch : ngrp + ch + 1],
                    )


            # -- remaining cheap groups -------------------------------------
            for gi, g in enumerate(groups[2:], start=2):
                load_group(gi, g)
                for m in range(MT):
                    cheap_unit(gi, g, m)

            na = ngrp + wch + 1
            for m in range(MT):
                nc.sync.dma_start(out=out[:, na * m : na * (m + 1)], in_=acc[m][:])
    _split_multi_waits(nc)
    return nc


_nc_by_cfg = {}


def _get_nc(wch):
    if wch not in _nc_by_cfg:
        _nc_by_cfg[wch] = _build_nc(wch)
    return _nc_by_cfg[wch]


def _fp8_cascade(x, n):
    """Split x into n fp8 rows summing (in f32) to ~x."""
    rows = []
    rem = np.asarray(x, np.float64).copy()
    for _ in range(n):
        h = rem.astype(FP8NP)
        rows.append(h)
        rem = rem - h.astype(np.float64)
    return rows


def _dr_tile(x):
    """[ncols, 512] fp8 -> [128, ncols/512 * 2048] in the DR chunk layout:
    [p, ch*2048 + (khat*2 + i)*512 + j] = x[512*ch + j, 256*khat + 128*i + p]."""
    nch = x.shape[0] // 512
    xt = np.ascontiguousarray(x.T)                  # [512, ncols]
    return np.ascontiguousarray(
        xt.reshape(2, 2, 128, nch, 512).transpose(2, 3, 0, 1, 4).reshape(128, -1)
    )


def _dr_tile_f(x):
    """[512 rows, 512 dims] fp8 -> [128, MT*512] stationary layout:
    [p, ((m*2 + khat)*2 + i)*128 + q] = x[128*m + q, 256*khat + 128*i + p]."""
    xt = np.ascontiguousarray(x.T)                  # [512 dims, 512 rows]
    return np.ascontiguousarray(
        xt.reshape(2, 2, 128, MT, 128).transpose(2, 3, 0, 1, 4).reshape(128, -1)
    )


def _prepare(centers1, features, targets, features_ood, pseudo_target_ood):
    """Host-side prep: sort rows by class, shard contiguously, and per core
    permute the g columns to [own 512 | matched | rest | ood | pad] so all
    eq-matches (and the diagonal, at window column 128m+p) land in the first
    WIN columns."""
    centers1 = np.asarray(centers1, np.float32)
    features = np.asarray(features, np.float32)
    features_ood = np.asarray(features_ood, np.float32)
    targets = np.asarray(targets).astype(np.int64)
    pseudo = np.asarray(pseudo_target_ood).astype(np.int64)

    tac = np.concatenate([targets, np.arange(C), pseudo])
    w_full = np.bincount(tac, minlength=C).astype(np.float64)

    # class-id label per g row (incl. centers/ood), and bias per g row
    lab = np.concatenate([targets, np.arange(C), np.full(BO, C, np.int64),
                          np.full(PAD, -1, np.int64)])
    bias1 = np.full(NPAD, -20.0, np.float64)
    bias1[:N] = -(np.log(w_full[tac]) + 10.0) / 10.0

    g = np.concatenate(
        [features, centers1, features_ood, np.zeros((PAD, D), np.float32)], axis=0
    )
    g8h = g.astype(FP8NP)
    g8l = (g - g8h.astype(np.float32)).astype(FP8NP)

    row_perm = np.argsort(targets, kind="stable")
    t_sorted = targets[row_perm]

    # per-core column permutations
    perms = []
    win_need = 1
    all_batch = np.arange(B)
    for c in range(NCORES):
        own = row_perm[RPC * c : RPC * (c + 1)]            # sorted by class
        tset = np.zeros(C + 1, bool)
        tset[t_sorted[RPC * c : RPC * (c + 1)]] = True
        in_own = np.zeros(B, bool)
        in_own[own] = True
        match_b = all_batch[tset[targets] & ~in_own]       # other cores' rows, own classes
        match_c = B + np.flatnonzero(tset[:C])             # centers of own classes
        matched = np.concatenate([match_b, match_c])
        rest_mask = np.ones(B + C, bool)
        rest_mask[own] = False
        rest_mask[matched] = False
        rest = np.flatnonzero(rest_mask)
        perm = np.concatenate(
            [own, matched, rest,
             np.arange(B + C, N),                          # ood
             np.arange(N, NPAD)]                           # pad
        )
        assert perm.shape == (NPAD,)
        perms.append(perm)
        win_need = max(win_need, RPC + len(matched))

    wch = max(2, -(-win_need // 512))
    win = 512 * wch
    cch = NCH - wch

    # window bias rows (bf16 hi+lo)
    bh_all = bias1.astype(BFNP)
    bl_all = (bias1 - bh_all.astype(np.float64)).astype(BFNP)
    b8_rows = _fp8_cascade(bias1, 3)                       # bh, bm, bl fp8

    ones8_host = np.zeros((2, 2, 128), np.float32)
    ones8_host[0, 0] = 1.0
    ones8_host[0, 1] = 1.0
    ones8_host[1, 0] = 1.0
    ones2_host = np.ones((2, 128), np.float32)
    ident = np.eye(128, dtype=np.float32)

    in_maps = []
    for c in range(NCORES):
        perm = perms[c]
        own = perm[:RPC]
        f8h = g8h[own]                                     # [512, 512] fp8
        f8l = g8l[own]
        # cheap bias rows: [p, cch_idx, i, j] with (0,0)=bh (0,1)=bm (1,0)=bl
        b8c = np.zeros((2, cch, 2, 512), FP8NP)
        pc = perm[win:].reshape(cch, 512)
        b8c[0, :, 0] = b8_rows[0][pc]
        b8c[0, :, 1] = b8_rows[1][pc]
        b8c[1, :, 0] = b8_rows[2][pc]
        bw_c = np.stack([bh_all[perm[:win]], bl_all[perm[:win]]])
        ta_p = lab[perm[:win]].astype(np.float32)
        in_maps.append(
            {
                "warm": np.full((2, 64), 0.125, FP8NP),
                "gT8": _dr_tile(g8h[perm]),
                "gT8l": _dr_tile(g8l[perm[:win]]),
                "fT8": _dr_tile_f(f8h),
                "fT8l": _dr_tile_f(f8l),
                "ones8": np.ascontiguousarray(ones8_host.reshape(2, 256).astype(FP8NP)),
                "b8": np.ascontiguousarray(b8c.reshape(2, cch * 1024)),
                "ones2": np.ascontiguousarray(ones2_host.astype(BFNP)),
                "bw": np.ascontiguousarray(bw_c.astype(BFNP)),
                "ta": np.ascontiguousarray(np.broadcast_to(ta_p, (128, win))),
                "tvec": np.ascontiguousarray(
                    t_sorted[RPC * c : RPC * (c + 1)].reshape(MT, 128).T.astype(np.float32)
                ),
                "ident": ident,
            }
        )

    # host-side numerator: P_i = 10*(f_i . G_{t_i} - f_i . f_i) - 10*K_i
    # with G_c = sum of batch features of class c + center_c (exact, f64)
    f64 = features.astype(np.float64)
    G = centers1.astype(np.float64).copy()
    np.add.at(G, targets, f64)
    dots = np.einsum("ij,ij->i", f64, G[targets])
    self_dot = np.einsum("ij,ij->i", f64, f64)
    P_half = dots - self_dot                       # sum over matched != self of r

    ncheap_groups = -(-(NCH - wch) // 4)
    host = {"t_sorted": t_sorted, "w_full": w_full, "wch": wch,
            "ngrp": ncheap_groups + 1, "P_half": P_half[row_perm]}
    return in_maps, host


def _combine(results, host):
    t_sorted = host["t_sorted"]
    w_full = host["w_full"]
    cnt_batch = np.bincount(t_sorted, minlength=C).astype(np.float64)

    ngrp = host["ngrp"]
    wch = host["wch"]
    na = ngrp + wch + 1
    A = np.empty(B)
    S2 = np.empty(B)
    e1s = np.empty(B)
    for c in range(NCORES):
        o = np.asarray(results[c]["out"], np.float64)
        for m in range(MT):
            rs = slice(RPC * c + 128 * m, RPC * c + 128 * (m + 1))
            a = o[:, na * m : na * (m + 1)]
            A[rs] = a[:, 0:ngrp].sum(axis=1)
            S2[rs] = a[:, ngrp : ngrp + wch].sum(axis=1)
            e1s[rs] = a[:, ngrp + wch]

    ws = w_full[t_sorted]
    K = cnt_batch[t_sorted]
    ds_ = 1.0 / (ws - 1.0) - 1.0 / ws
    S = A - e1s + ds_ * ws * (S2 - e1s)
    P = 10.0 * host["P_half"] - 10.0 * K
    val = P / K - np.log(S)
    return np.float32(-val.mean())


def _run(inputs, trace=False, **kw):
    in_maps, host = _prepare(**inputs)
    nc = _get_nc(host["wch"])
    res = run_bass_kernel_spmd(nc, in_maps, list(range(NCORES)), trace=trace, **kw)
    loss = _combine(res.results, host)
    return loss, res


def kernel(**inputs):
    loss, _ = _run(inputs)
    return loss


# revision 20
# speedup vs baseline: 1.7637x; 1.0052x over previous
"""Trainium2 Bass kernel for the class-balanced supervised-contrastive loss.

Math (reference semantics, shift-invariant form with constant shift 10):
  l_ij = (f_i . g_j) / T,  T = 0.1, g = [features; centers; features_ood]
  E_ij = exp(l_ij - 10)
  S_i  = sum_{j != i} E_ij / (w_j - eq_ij)        (w_j = class count, eq = label match)
  P_i  = sum_{j != i} eq_ij (l_ij - 10)
  loss = -mean_i( P_i / K_i - log S_i ),  K_i = batch count of class t_i

Device per core (rows globally sorted by class, 512 rows/core, columns permuted
so every eq-match lands in the first WIN=1024 cols):
  psum = f . g + bias1_col   (bias1 = (ln(1/w) - 10)/10, so exp(10*psum) = E/w)
  A_i  = sum_j exp(10*psum)            ACT exp accum_out
  S2_i = sum_{win} eq * E1             DVE masked reduce (window only)
  S3_i = sum_{win} eq * psum           DVE masked reduce (window only)
  diag = psum_ii                       self column, for exclusion terms

All matmuls run as fp8e4 DoubleRow (2 contraction rows per PE pass):
  - main terms: f8h . g8h over K=512 as 2 DR passes of K_eff=256
  - window adds f8l.g8h + f8h.g8l correction terms (dot err ~2.5e-4) plus an
    exact bf16 (hi,lo) K=2 bias matmul
  - cheap (non-window) cols get their bias as a 3-row fp8 DR matmul
    (bh,bm,bl residual cascade, exponent err ~1e-2 -> A err ~1e-4), which is
    also the bank-opening start=True instruction for psum zeroing
Column space per (m-tile) is processed in [128,2048] psum groups (4 banks,
double buffered), each consumed by one wide ACT exp with accum_out.
"""

import ml_dtypes
import numpy as np

import concourse.bass as bass
import concourse.mybir as mybir
import concourse.tile as tile
from concourse.bass_utils import run_bass_kernel_spmd

NCORES = 8
C, TEMP = 1000, 0.1
B, BO, D = 4096, 4096, 512
N = B + C + BO              # 9192
NPAD = 9216                 # 18 * 512
PAD = NPAD - N
NCH = NPAD // 512           # 18 column chunks
RPC = B // NCORES           # 512 rows per core
MT = RPC // 128             # 4 row tiles per core

F32 = mybir.dt.float32
BF16 = mybir.dt.bfloat16
FP8 = mybir.dt.float8e4
DR = mybir.MatmulPerfMode.DoubleRow
ALU = mybir.AluOpType
AF = mybir.ActivationFunctionType
BFNP = ml_dtypes.bfloat16
FP8NP = ml_dtypes.float8_e4m3

# This walrus build accepts only one sync-wait command per engine instruction.
# Move surplus waits onto standalone EventSemaphore instructions just before
# the affected instruction (same engine, so blocking semantics are identical).
_SPLIT_SKIP = ("InstEventSemaphore",)


def _split_multi_waits(nc):
    n = 0
    for f in nc.m.functions:
        for bb in f.blocks:
            new = []
            for ins in bb.instructions:
                si = ins.sync_info
                if (
                    si is not None
                    and si.on_wait
                    and len(si.on_wait) > 1
                    and type(ins).__name__ not in _SPLIT_SKIP
                ):
                    waits = list(si.on_wait)
                    for w in waits[:-1]:
                        es = mybir.InstEventSemaphore(
                            name=f"wsplit_{n}",
                            engine=ins.engine,
                            sync_info=mybir.SyncInfo(on_wait=[w], on_update=[]),
                        )
                        n += 1
                        new.append(es)
                    ins.sync_info = mybir.SyncInfo(
                        on_wait=[waits[-1]], on_update=list(si.on_update)
                    )
                new.append(ins)
            bb.instructions = new
    return n


def _build_nc(wch=2):
    """wch = number of 512-col window chunks holding all eq-matches."""
    cch = NCH - wch                     # cheap chunks
    win = 512 * wch
    # cheap chunks packed into psum groups of <=4 chunks
    groups = []
    ch = wch
    while ch < NCH:
        g = list(range(ch, min(ch + 4, NCH)))
        groups.append(g)
        ch += len(g)
    ngrp = len(groups) + 1              # + window group
    nc = bass.Bass()

    # DR layouts: contraction row r = 256*khat + 128*i + p  (pair i, partition p)
    gT8 = nc.declare_dram_parameter("gT8", [128, NCH * 2048], FP8, isOutput=False)
    gT8l = nc.declare_dram_parameter("gT8l", [128, wch * 2048], FP8, isOutput=False)
    fT8 = nc.declare_dram_parameter("fT8", [128, MT * 512], FP8, isOutput=False)
    fT8l = nc.declare_dram_parameter("fT8l", [128, MT * 512], FP8, isOutput=False)
    warm = nc.declare_dram_parameter("warm", [2, 64], FP8, isOutput=False)
    ones8 = nc.declare_dram_parameter("ones8", [2, 256], FP8, isOutput=False)
    b8 = nc.declare_dram_parameter("b8", [2, cch * 1024], FP8, isOutput=False)
    ones2 = nc.declare_dram_parameter("ones2", [2, 128], BF16, isOutput=False)
    bw = nc.declare_dram_parameter("bw", [2, win], BF16, isOutput=False)
    ta = nc.declare_dram_parameter("ta", [128, win], F32, isOutput=False)
    tvec = nc.declare_dram_parameter("tvec", [128, MT], F32, isOutput=False)
    ident = nc.declare_dram_parameter("ident", [128, 128], F32, isOutput=False)
    out = nc.declare_dram_parameter("out", [128, (len(groups) + 1 + wch + 1) * MT], F32, isOutput=True)

    with tile.TileContext(nc) as tc:
        with (
            tc.tile_pool(name="const", bufs=1) as const,
            tc.tile_pool(name="stats", bufs=1) as stats,
            tc.tile_pool(name="gt", bufs=8) as gtp,
            tc.tile_pool(name="e1c", bufs=3) as e1cp,
            tc.tile_pool(name="e1w", bufs=2) as e1wp,
            tc.tile_pool(name="psum", bufs=2, space="PSUM") as psp,
        ):
            ft = const.tile([128, MT, 2, 2, 128], FP8)
            ftl = const.tile([128, MT, 2, 2, 128], FP8)
            warm_sb = const.tile([2, 2, 32], FP8)
            warm_o = const.tile([2, 32], F32)
            ones8_sb = const.tile([2, 2, 128], FP8)
            b8_sb = const.tile([2, cch, 2, 512], FP8)
            ones2_sb = const.tile([2, 128], BF16)
            bw_sb = const.tile([2, win], BF16)
            ta_sb = const.tile([128, win], F32)
            tvec_sb = const.tile([128, MT], F32)
            ident_sb = const.tile([128, 128], F32)
            gl = const.tile([128, wch, 2, 2, 512], FP8)

            nc.sync.dma_start(out=warm_sb[:], in_=warm[:])
            nc.scalar.dma_start(out=ones8_sb[:], in_=ones8[:])
            nc.gpsimd.dma_start(out=ft[:], in_=fT8[:])
            # Exp table preload off the critical path
            nc.scalar.activation(warm_o[:], warm_sb[:, 0], AF.Exp, scale=1.0)

            # per m: ngrp A-partials | wch S2 parts | e1s diag
            acc = [stats.tile([128, ngrp + wch + 1], F32, name=f"acc{m}") for m in range(MT)]

            group_tiles = {}

            def load_group(gi, g, spread=False):
                beng = nc.sync
                beng.dma_start(
                    out=b8_sb[:, g[0] - wch : g[-1] + 1 - wch],
                    in_=b8[:, 1024 * (g[0] - wch) : 1024 * (g[-1] + 1 - wch)],
                )
                gts = []
                for ci, ch in enumerate(g):
                    gt = gtp.tile([128, 2, 2, 512], FP8, name=f"g{ch}", tag="gt")
                    eng = nc.sync
                    if spread:
                        eng = (nc.gpsimd, nc.gpsimd, nc.sync, nc.gpsimd)[ci % 4]
                    eng.dma_start(
                        out=gt[:], in_=gT8[:, 2048 * ch : 2048 * (ch + 1)]
                    )
                    gts.append(gt)
                group_tiles[gi] = gts

            def cheap_unit(gi, g, m, warmups=0):
                gts = group_tiles[gi]
                ps = psp.tile([128, 2048], F32)
                # PE warmup/p-state ramp spins while the first DMAs land
                for _ in range(warmups):
                    nc.tensor.matmul(
                        ps[:, 0:128], ones8_sb[:], ones8_sb[:],
                        start=True, stop=True, perf_mode=DR, skip_group_check=True,
                    )
                for ci, ch in enumerate(g):
                    cs = slice(512 * ci, 512 * (ci + 1))
                    nc.tensor.matmul(
                        ps[:, cs], ones8_sb[:], b8_sb[:, ch - wch],
                        start=True, stop=False, perf_mode=DR,
                    )
                    for k in range(2):
                        nc.tensor.matmul(
                            ps[:, cs], ft[:, m, k], gts[ci][:, k],
                            start=False, stop=(k == 1), perf_mode=DR,
                        )
                e1c = e1cp.tile([128, 512 * len(g)], BF16, tag="e1c")
                nc.scalar.activation(
                    e1c[:], ps[:, : 512 * len(g)], AF.Exp, scale=10.0,
                    accum_out=acc[m][:, 1 + gi : 2 + gi],
                )

            load_group(0, groups[0], spread=True)
            cheap_unit(0, groups[0], 0, warmups=2)
            for m in range(1, MT):
                cheap_unit(0, groups[0], m)

            # window DMAs land while group 0/1 compute
            nc.scalar.dma_start(out=ftl[:], in_=fT8l[:])
            nc.scalar.dma_start(out=ones2_sb[:], in_=ones2[:])
            nc.scalar.dma_start(out=bw_sb[:], in_=bw[:])
            nc.gpsimd.dma_start(out=ta_sb[:], in_=ta[:])
            nc.gpsimd.dma_start(out=tvec_sb[:], in_=tvec[:])
            nc.gpsimd.dma_start(out=ident_sb[:], in_=ident[:])
            gw = []
            for ch in range(wch):
                nc.gpsimd.dma_start(
                    out=gl[:, ch], in_=gT8l[:, 2048 * ch : 2048 * (ch + 1)]
                )
                gt = gtp.tile([128, 2, 2, 512], FP8, name=f"gw{ch}", tag="gtw")
                nc.gpsimd.dma_start(out=gt[:], in_=gT8[:, 2048 * ch : 2048 * (ch + 1)])
                gw.append(gt)

            load_group(1, groups[1])
            for m in range(MT):
                # interleave: cheap unit first so its psum fill leads the pair
                cheap_unit(1, groups[1], m)
                ps = psp.tile([128, 2048], F32)
                for ch in range(wch):
                    cs = slice(512 * ch, 512 * (ch + 1))
                    nc.tensor.matmul(
                        ps[:, cs], ones2_sb[:], bw_sb[:, cs],
                        start=True, stop=False,
                    )
                    terms = (
                        (ft, (gw[ch][:, 0], gw[ch][:, 1])),
                        (ftl, (gw[ch][:, 0], gw[ch][:, 1])),
                        (ft, (gl[:, ch, 0], gl[:, ch, 1])),
                    )
                    for ti, (lhs, rhss) in enumerate(terms):
                        for k in range(2):
                            nc.tensor.matmul(
                                ps[:, cs], lhs[:, m, k], rhss[k],
                                start=False,
                                stop=(ti == 2 and k == 1),
                                perf_mode=DR,
                            )
                e1w = e1wp.tile([128, win], F32, tag="e1w")
                nc.scalar.activation(e1w[:], ps[:, :win], AF.Exp, scale=10.0)
                nc.vector.tensor_reduce(
                    acc[m][:, 0:1], e1w[:], mybir.AxisListType.X, ALU.add,
                )
                # self term e1s: gather the diagonal of the SBUF exp tile (no
                # psum read, so the psum buffer frees as soon as the exp ends)
                sd = e1wp.tile([128, 128], F32, tag="scrd")
                nc.vector.scalar_tensor_tensor(
                    out=sd[:], in0=ident_sb[:], scalar=1.0,
                    in1=e1w[:, 128 * m : 128 * (m + 1)],
                    op0=ALU.mult, op1=ALU.mult,
                    accum_out=acc[m][:, ngrp + wch : ngrp + wch + 1],
                )
                for ch in range(wch):
                    cs = slice(512 * ch, 512 * (ch + 1))
                    sc = e1wp.tile([128, 512], F32, tag="scr2")
                    nc.vector.scalar_tensor_tensor(
                        out=sc[:], in0=ta_sb[:, cs], scalar=tvec_sb[:, m : m + 1],
                        in1=e1w[:, cs], op0=ALU.is_equal, op1=ALU.mult,
                        accum_out=acc[m][:, ngrp + ch : ngrp + ch + 1],
                    )


            # -- remaining cheap groups -------------------------------------
            for gi, g in enumerate(groups[2:], start=2):
                load_group(gi, g)
                for m in range(MT):
                    cheap_unit(gi, g, m)

            na = ngrp + wch + 1
            for m in range(MT):
                nc.sync.dma_start(out=out[:, na * m : na * (m + 1)], in_=acc[m][:])
    _split_multi_waits(nc)
    return nc


_nc_by_cfg = {}


def _get_nc(wch):
    if wch not in _nc_by_cfg:
        _nc_by_cfg[wch] = _build_nc(wch)
    return _nc_by_cfg[wch]


def _fp8_cascade(x, n):
    """Split x into n fp8 rows summing (in f32) to ~x."""
    rows = []
    rem = np.asarray(x, np.float64).copy()
    for _ in range(n):
        h = rem.astype(FP8NP)
        rows.append(h)
        rem = rem - h.astype(np.float64)
    return rows


def _dr_tile(x):
    """[ncols, 512] fp8 -> [128, ncols/512 * 2048] in the DR chunk layout:
    [p, ch*2048 + (khat*2 + i)*512 + j] = x[512*ch + j, 256*khat + 128*i + p]."""
    nch = x.shape[0] // 512
    xt = np.ascontiguousarray(x.T)                  # [512, ncols]
    return np.ascontiguousarray(
        xt.reshape(2, 2, 128, nch, 512).transpose(2, 3, 0, 1, 4).reshape(128, -1)
    )


def _dr_tile_f(x):
    """[512 rows, 512 dims] fp8 -> [128, MT*512] stationary layout:
    [p, ((m*2 + khat)*2 + i)*128 + q] = x[128*m + q, 256*khat + 128*i + p]."""
    xt = np.ascontiguousarray(x.T)                  # [512 dims, 512 rows]
    return np.ascontiguousarray(
        xt.reshape(2, 2, 128, MT, 128).transpose(2, 3, 0, 1, 4).reshape(128, -1)
    )


def _prepare(centers1, features, targets, features_ood, pseudo_target_ood):
    """Host-side prep: sort rows by class, shard contiguously, and per core
    permute the g columns to [own 512 | matched | rest | ood | pad] so all
    eq-matches (and the diagonal, at window column 128m+p) land in the first
    WIN columns."""
    centers1 = np.asarray(centers1, np.float32)
    features = np.asarray(features, np.float32)
    features_ood = np.asarray(features_ood, np.float32)
    targets = np.asarray(targets).astype(np.int64)
    pseudo = np.asarray(pseudo_target_ood).astype(np.int64)

    tac = np.concatenate([targets, np.arange(C), pseudo])
    w_full = np.bincount(tac, minlength=C).astype(np.float64)

    # class-id label per g row (incl. centers/ood), and bias per g row
    lab = np.concatenate([targets, np.arange(C), np.full(BO, C, np.int64),
                          np.full(PAD, -1, np.int64)])
    bias1 = np.full(NPAD, -20.0, np.float64)
    bias1[:N] = -(np.log(w_full[tac]) + 10.0) / 10.0

    g = np.concatenate(
        [features, centers1, features_ood, np.zeros((PAD, D), np.float32)], axis=0
    )
    g8h = g.astype(FP8NP)
    g8l = (g - g8h.astype(np.float32)).astype(FP8NP)

    row_perm = np.argsort(targets, kind="stable")
    t_sorted = targets[row_perm]

    # per-core column permutations
    perms = []
    win_need = 1
    all_batch = np.arange(B)
    for c in range(NCORES):
        own = row_perm[RPC * c : RPC * (c + 1)]            # sorted by class
        tset = np.zeros(C + 1, bool)
        tset[t_sorted[RPC * c : RPC * (c + 1)]] = True
        in_own = np.zeros(B, bool)
        in_own[own] = True
        match_b = all_batch[tset[targets] & ~in_own]       # other cores' rows, own classes
        match_c = B + np.flatnonzero(tset[:C])             # centers of own classes
        matched = np.concatenate([match_b, match_c])
        rest_mask = np.ones(B + C, bool)
        rest_mask[own] = False
        rest_mask[matched] = False
        rest = np.flatnonzero(rest_mask)
        perm = np.concatenate(
            [own, matched, rest,
             np.arange(B + C, N),                          # ood
             np.arange(N, NPAD)]                           # pad
        )
        assert perm.shape == (NPAD,)
        perms.append(perm)
        win_need = max(win_need, RPC + len(matched))

    wch = max(2, -(-win_need // 512))
    win = 512 * wch
    cch = NCH - wch

    # window bias rows (bf16 hi+lo)
    bh_all = bias1.astype(BFNP)
    bl_all = (bias1 - bh_all.astype(np.float64)).astype(BFNP)
    b8_rows = _fp8_cascade(bias1, 3)                       # bh, bm, bl fp8

    ones8_host = np.zeros((2, 2, 128), np.float32)
    ones8_host[0, 0] = 1.0
    ones8_host[0, 1] = 1.0
    ones8_host[1, 0] = 1.0
    ones2_host = np.ones((2, 128), np.float32)
    ident = np.eye(128, dtype=np.float32)

    in_maps = []
    for c in range(NCORES):
        perm = perms[c]
        own = perm[:RPC]
        f8h = g8h[own]                                     # [512, 512] fp8
        f8l = g8l[own]
        # cheap bias rows: [p, cch_idx, i, j] with (0,0)=bh (0,1)=bm (1,0)=bl
        b8c = np.zeros((2, cch, 2, 512), FP8NP)
        pc = perm[win:].reshape(cch, 512)
        b8c[0, :, 0] = b8_rows[0][pc]
        b8c[0, :, 1] = b8_rows[1][pc]
        b8c[1, :, 0] = b8_rows[2][pc]
        bw_c = np.stack([bh_all[perm[:win]], bl_all[perm[:win]]])
        ta_p = lab[perm[:win]].astype(np.float32)
        in_maps.append(
            {
                "warm": np.full((2, 64), 0.125, FP8NP),
                "gT8": _dr_tile(g8h[perm]),
                "gT8l": _dr_tile(g8l[perm[:win]]),
                "fT8": _dr_tile_f(f8h),
                "fT8l": _dr_tile_f(f8l),
                "ones8": np.ascontiguousarray(ones8_host.reshape(2, 256).astype(FP8NP)),
                "b8": np.ascontiguousarray(b8c.reshape(2, cch * 1024)),
                "ones2": np.ascontiguousarray(ones2_host.astype(BFNP)),
                "bw": np.ascontiguousarray(bw_c.astype(BFNP)),
                "ta": np.ascontiguousarray(np.broadcast_to(ta_p, (128, win))),
                "tvec": np.ascontiguousarray(
                    t_sorted[RPC * c : RPC * (c + 1)].reshape(MT, 128).T.astype(np.float32)
                ),
                "ident": ident,
            }
        )

    # host-side numerator: P_i = 10*(f_i . G_{t_i} - f_i . f_i) - 10*K_i
    # with G_c = sum of batch features of class c + center_c (exact, f64)
    f64 = features.astype(np.float64)
    G = centers1.astype(np.float64).copy()
    np.add.at(G, targets, f64)
    dots = np.einsum("ij,ij->i", f64, G[targets])
    self_dot = np.einsum("ij,ij->i", f64, f64)
    P_half = dots - self_dot                       # sum over matched != self of r

    ncheap_groups = -(-(NCH - wch) // 4)
    host = {"t_sorted": t_sorted, "w_full": w_full, "wch": wch,
            "ngrp": ncheap_groups + 1, "P_half": P_half[row_perm]}
    return in_maps, host


def _combine(results, host):
    t_sorted = host["t_sorted"]
    w_full = host["w_full"]
    cnt_batch = np.bincount(t_sorted, minlength=C).astype(np.float64)

    ngrp = host["ngrp"]
    wch = host["wch"]
    na = ngrp + wch + 1
    A = np.empty(B)
    S2 = np.empty(B)
    e1s = np.empty(B)
    for c in range(NCORES):
        o = np.asarray(results[c]["out"], np.float64)
        for m in range(MT):
            rs = slice(RPC * c + 128 * m, RPC * c + 128 * (m + 1))
            a = o[:, na * m : na * (m + 1)]
            A[rs] = a[:, 0:ngrp].sum(axis=1)
            S2[rs] = a[:, ngrp : ngrp + wch].sum(axis=1)
            e1s[rs] = a[:, ngrp + wch]

    ws = w_full[t_sorted]
    K = cnt_batch[t_sorted]
    ds_ = 1.0 / (ws - 1.0) - 1.0 / ws
    S = A - e1s + ds_ * ws * (S2 - e1s)
    P = 10.0 * host["P_half"] - 10.0 * K
    val = P / K - np.log(S)
    return np.float32(-val.mean())


def _run(inputs, trace=False, **kw):
    in_maps, host = _prepare(**inputs)
    nc = _get_nc(host["wch"])
    res = run_bass_kernel_spmd(nc, in_maps, list(range(NCORES)), trace=trace, **kw)
    loss = _combine(res.results, host)
    return loss, res


def kernel(**inputs):
    loss, _ = _run(inputs)
    return loss
